# revision 17
# baseline (speedup 1.0000x reference)
"""V3: class-half sharded GNN kernel, gather-free stage A.

Core (b, h) owns batch b and CLASS-half h: the nodes whose own class
(c2n_row) falls in half h.  Stage A needs no AllReduce: each core computes
complete class sums for its half from host-presorted contiguous bf16 rows
(class-block padded) via indicator matmuls, normalizes, and writes a bf16
paired-class half-table; a pairwise AllGather concatenates the halves.
Stage B gathers paired-class rows per edge (dma_gather, deep-buffered so
the ~2.2us/call cadence is not consumer-stalled) and indicator-matmuls
node-context blocks; fused LayerNorm MLP with Rsqrt activation.
"""

import numpy as np

SCR = 128
H = 64
LN_EPS = 1e-5


def _ru(x, m):
    return (x + m - 1) // m * m


def _wrap16(idx):
    n = len(idx)
    n16 = _ru(n, 16)
    a = np.full(n16, -1, dtype=np.int16)
    a[:n] = idx
    a = a.reshape(n16 // 16, 16).T
    return np.tile(a, (8, 1)).copy()


def _cols128(vals, dtype=np.float32):
    n = len(vals)
    return np.asarray(vals, dtype=dtype).reshape(n // 128, 128).T.copy()


def make_cfg(B, N, C, E, tg=1024):
    cfg = dict(B=B, N=N, C=C, E=E, n_cores=2 * B, TG=tg)
    cfg["GROUP"] = 512
    cfg["CPAD"] = _ru(C, 256)
    cfg["CH"] = cfg["CPAD"] // 2          # classes per half
    cfg["QH"] = cfg["CH"] // 2            # paired-class rows per half
    cfg["QT"] = 2 * cfg["QH"]
    cfg["CBH"] = cfg["CH"] // 128         # class blocks per half
    cfg["RC2"] = SCR + cfg["QT"] + SCR    # gather table rows
    assert SCR + cfg["QT"] < 32768
    return cfg


def host_prep(cfg, inputs):
    N, C, CH = cfg["N"], cfg["C"], cfg["CH"]
    TG, CBH = cfg["TG"], cfg["CBH"]
    c2n_row = np.asarray(inputs["c2n_row"]).astype(np.int64)
    n2c_row = np.asarray(inputs["n2c_row"]).astype(np.int64)
    n2c_col = np.asarray(inputs["n2c_col"]).astype(np.int64)

    cnt_c = np.bincount(c2n_row, minlength=C).astype(np.float32)
    invc = (1.0 / np.maximum(cnt_c, 1.0)).astype(np.float32)
    cnt_n = np.bincount(n2c_row, minlength=N).astype(np.float32)
    invn = (1.0 / np.maximum(cnt_n, 1.0)).astype(np.float32)

    # ---------- node streams per class-half
    nodes_h, order_h = {}, {}
    for h in (0, 1):
        sel = np.nonzero((c2n_row >= h * CH) & (c2n_row < (h + 1) * CH))[0]
        order = np.argsort(c2n_row[sel], kind="stable")
        nodes_h[h] = sel[order]           # original node ids, class-sorted
    n_h = {h: len(nodes_h[h]) for h in (0, 1)}
    NTOK = _ru(max(n_h.values()), 1024)
    cfg["NTOK"] = NTOK
    cfg["NGRP"] = NTOK // cfg["GROUP"]
    NBLK = NTOK // 128
    cfg["NBLK"] = NBLK

    # ---------- stage A caps (shared): members per class block
    capsA = np.zeros(CBH, dtype=np.int64)
    for h in (0, 1):
        cls_loc = c2n_row[nodes_h[h]] - h * CH
        capsA = np.maximum(capsA, np.bincount(cls_loc // 128, minlength=CBH))
    capsA = _ru(np.maximum(capsA, 1), 128)
    TA = int(capsA.sum())
    TA_pad = _ru(TA, TG)
    schedA = []
    for cb in range(CBH):
        for k in range(capsA[cb] // 128):
            schedA.append((cb, k == 0))
    for _ in range((TA_pad - TA) // 128):
        schedA.append((-1, False))
    cfg["TA"] = TA_pad

    # ---------- stage B caps: edges per (node block, parity)
    edges_h = {}
    capsB = np.zeros((NBLK, 2), dtype=np.int64)
    for h in (0, 1):
        pos = np.full(N, -1, dtype=np.int64)
        pos[nodes_h[h]] = np.arange(n_h[h])
        sel = np.nonzero(pos[n2c_row] >= 0)[0]
        dst = pos[n2c_row[sel]]
        col = n2c_col[sel]
        par = col % 2
        order = np.lexsort((col, par, dst // 128))
        edges_h[h] = (dst[order], col[order], par[order])
        nbk = dst[order] // 128
        for prt in (0, 1):
            cnt = np.bincount(nbk[par[order] == prt], minlength=NBLK)
            capsB[:, prt] = np.maximum(capsB[:, prt], cnt)
    capsB = _ru(np.maximum(capsB, 1), 128)
    TBn = int(capsB.sum())
    TB = _ru(TBn, TG)
    cfg["TB"] = TB
    schedB = []
    for nb in range(NBLK):
        for prt in (0, 1):
            for k in range(capsB[nb, prt] // 128):
                schedB.append((nb, prt))
    for _ in range((TB - TBn) // 128):
        schedB.append((-1, 0))

    # ---------- per-half index arrays
    pre = {}
    for h in (0, 1):
        d = {}
        # stage A: padded row placement + segids
        cls_loc = c2n_row[nodes_h[h]] - h * CH
        segA = np.full(cfg["TA"], 255, dtype=np.float32)
        rowsrc = np.full(cfg["TA"], -1, dtype=np.int64)  # index into node stream
        cnts = np.bincount(cls_loc // 128, minlength=CBH)
        starts = np.r_[0, np.cumsum(cnts)]
        base = 0
        for cb in range(CBH):
            nmem = int(cnts[cb])
            s = starts[cb]
            rowsrc[base:base + nmem] = np.arange(s, s + nmem)
            segA[base:base + nmem] = cls_loc[s:s + nmem] - 128 * cb
            base += capsA[cb]
        import ml_dtypes
        d["rowsrcA"] = rowsrc
        d["segA"] = _cols128(segA, ml_dtypes.bfloat16)

        # stage B
        dst, col, par = edges_h[h]
        nbk = dst // 128
        gidx = np.zeros(TB, dtype=np.int64)
        segB = np.full(TB, 255, dtype=np.float32)
        wgtB = np.zeros(TB, dtype=np.float32)
        base = 0
        for nb in range(NBLK):
            for prt in (0, 1):
                m = (nbk == nb) & (par == prt)
                nmem = int(m.sum())
                gidx[base:base + nmem] = SCR + col[m] // 2
                segB[base:base + nmem] = dst[m] - 128 * nb
                wgtB[base:base + nmem] = invn[nodes_h[h][dst[m]]]
                base += capsB[nb, prt]
        import ml_dtypes
        bf = ml_dtypes.bfloat16
        d["gidxB"] = _wrap16(gidx)
        d["segB"] = _cols128(segB, bf)
        d["wgtB"] = _cols128(wgtB, bf)
        pre[h] = d

    meta = dict(schedA=schedA, schedB=schedB)
    return pre, meta, dict(invc=invc, nodes_h=nodes_h, n_h=n_h)


def build(cfg, meta, wvals):
    from concourse import bass, bacc, tile, mybir

    f32 = mybir.dt.float32
    bf16 = mybir.dt.bfloat16
    i16 = mybir.dt.int16
    AF = mybir.ActivationFunctionType
    ALU = mybir.AluOpType

    CH, QH, QT = cfg["CH"], cfg["QH"], cfg["QT"]
    CBH, RC2 = cfg["CBH"], cfg["RC2"]
    NTOK, TG, GROUP = cfg["NTOK"], cfg["TG"], cfg["GROUP"]
    NBLK, TA, TB = cfg["NBLK"], cfg["TA"], cfg["TB"]
    n_cores = cfg["n_cores"]
    schedA, schedB = meta["schedA"], meta["schedB"]
    KC = TG // 128

    nc = bacc.Bacc("TRN2", target_bir_lowering=False, debug=False,
                   num_devices=n_cores, num_swdge_queues=4)

    embA = nc.dram_tensor("embA", [TA, H], bf16, kind="ExternalInput")
    segA = nc.dram_tensor("segA", [128, TA // 128], bf16, kind="ExternalInput")
    embN = nc.dram_tensor("embN", [NTOK, H], bf16, kind="ExternalInput")
    gidxB = nc.dram_tensor("gidxB", [128, TB // 16], i16, kind="ExternalInput")
    segB = nc.dram_tensor("segB", [128, TB // 128], bf16, kind="ExternalInput")
    wgtB = nc.dram_tensor("wgtB", [128, TB // 128], bf16, kind="ExternalInput")
    invc_tok = nc.dram_tensor("invc_tok", [128, CBH], f32,
                              kind="ExternalInput")
    iota_d = nc.dram_tensor("iota_d", [128, 128], bf16, kind="ExternalInput")
    w1blk = nc.dram_tensor("w1blk", [128, 128], bf16, kind="ExternalInput")
    b1col = nc.dram_tensor("b1col", [128, 1], f32, kind="ExternalInput")
    stats_lhsT = nc.dram_tensor("stats_lhsT", [128, 2], bf16,
                                kind="ExternalInput")
    gamma2 = nc.dram_tensor("gamma2", [128, 128], bf16, kind="ExternalInput")
    beta2 = nc.dram_tensor("beta2", [128, 128], bf16, kind="ExternalInput")
    w2col = nc.dram_tensor("w2col", [128, 2], bf16, kind="ExternalInput")
    identd = nc.dram_tensor("identd", [128, 128], bf16, kind="ExternalInput")
    out = nc.dram_tensor("out", [cfg["NGRP"], GROUP], f32,
                         kind="ExternalOutput")

    chalf = nc.dram_tensor("chalf", [QH, 128], bf16)
    cemb2 = nc.dram_tensor("cemb2", [RC2, 128], bf16)
    cembg = nc.dram_tensor("cembg", [RC2, 128], bf16)

    b2v = float(wvals["b2"])

    with tile.TileContext(nc) as tc:
        # ================= stage A =================
        with tc.tile_pool(name="pA", bufs=1) as pA, \
             tc.tile_pool(name="pAg", bufs=4) as pAg, \
             tc.tile_pool(name="pAn", bufs=3) as pAn, \
             tc.tile_pool(name="pAp", bufs=4, space="PSUM") as pAp:
            csum_sb = pA.tile([128, CBH, H], f32)
            sa = pA.tile([128, TA // 128], bf16)
            iota_s = pA.tile([128, 128], bf16)
            iv = pA.tile([128, CBH], f32)
            zb = pA.tile([128, SCR], bf16)
            nc.scalar.dma_start(sa[:], segA[:, :])
            nc.scalar.dma_start(iota_s[:], iota_d[:, :])
            nc.scalar.dma_start(iv[:], invc_tok[:, :])
            nc.vector.memset(csum_sb[:], 0.0)
            nc.vector.memset(zb[:], 0.0)
            # zero guard rows of the gather table
            nc.scalar.dma_start(bass.AP(cemb2, 0, [[128, SCR], [1, 128]]),
                                zb[:])

            cur_tile = None
            cur_ind = None
            pend = []
            prev_cb = None

            def flushA(pend, cb):
                if not pend:
                    return
                ps = pAp.tile([128, H], f32, tag="psA")
                for i, (tl, ind_t, kk) in enumerate(pend):
                    nc.tensor.matmul(ps[:], ind_t[:, kk, :], tl[:, kk, :],
                                     start=(i == 0),
                                     stop=(i == len(pend) - 1))
                nc.vector.tensor_add(csum_sb[:, cb, :],
                                     csum_sb[:, cb, :], ps[:])

            for ck, (cb, first) in enumerate(schedA):
                call = ck // KC
                if ck % KC == 0:
                    cur_tile = pAg.tile([128, KC, H], bf16, tag="gA")
                    nc.scalar.dma_start(
                        cur_tile[:],
                        bass.AP(embA, call * TG * H,
                                [[H, 128], [128 * H, KC], [1, H]]))
                    cur_ind = pAg.tile([128, KC, 128], bf16, tag="indA")
                    nc.vector.tensor_tensor(
                        cur_ind[:],
                        iota_s[:].unsqueeze(1).broadcast_to([128, KC, 128]),
                        (sa[:, call * KC:(call + 1) * KC]
                         .unsqueeze(2).broadcast_to([128, KC, 128])),
                        ALU.is_equal)
                if cb < 0:
                    continue
                if cb != prev_cb:
                    flushA(pend, prev_cb)
                    pend = []
                    prev_cb = cb
                pend.append((cur_tile, cur_ind, ck % KC))
            flushA(pend, prev_cb)

            # normalize -> bf16 paired half table (class c at elems c*64)
            NBS = 32   # blocks per normalize slab
            cb0 = 0
            while cb0 < CBH:
                nb_ = min(NBS, CBH - cb0)
                tbf = pAn.tile([128, NBS, H], bf16, tag="nrmb")
                ivb = (iv[:, cb0:cb0 + nb_]
                       .unsqueeze(2).broadcast_to([128, nb_, H]))
                nc.vector.tensor_mul(tbf[:, :nb_, :],
                                     csum_sb[:, cb0:cb0 + nb_, :], ivb)
                nc.scalar.dma_start(
                    bass.AP(chalf, cb0 * 128 * H,
                            [[H, 128], [128 * H, nb_], [1, H]]),
                    tbf[:, :nb_, :])
                cb0 += nb_

        # ================= AllGather halves =================
        groups = [[2 * b, 2 * b + 1] for b in range(n_cores // 2)]
        nc.gpsimd.collective_compute(
            "AllGather", mybir.AluOpType.bypass,
            replica_groups=groups,
            ins=[bass.AP(chalf, 0, [[128, QH], [1, 128]]).opt()],
            outs=[bass.AP(cemb2, SCR * 128, [[128, QT], [1, 128]]).opt()])

        # copy AG output out of the collectives pool (random reads there
        # are ~4x slower) into a plain internal tensor for the gathers
        with tc.tile_pool(name="pcp", bufs=2) as pcp:
            CPR = 4224   # rows per copy slab; RC2 = 198*128
            r0 = 0
            while r0 < RC2:
                rr = min(CPR, RC2 - r0)
                nb_ = rr // 128
                tcp = pcp.tile([128, CPR // 128, 128], bf16, tag="cp")
                nc.scalar.dma_start(
                    tcp[:, :nb_, :],
                    bass.AP(cemb2, r0 * 128,
                            [[128, 128], [128 * 128, nb_], [1, 128]]))
                nc.scalar.dma_start(
                    bass.AP(cembg, r0 * 128,
                            [[128, 128], [128 * 128, nb_], [1, 128]]),
                    tcp[:, :nb_, :])
                r0 += rr

        # ================= stage B + MLP =================
        with tc.tile_pool(name="pw", bufs=1) as pw:
            w1_s = pw.tile([128, 128], bf16)
            b1_s = pw.tile([128, 1], f32)
            st_s = pw.tile([128, 2], bf16)
            ga_s = pw.tile([128, 128], bf16)
            be_s = pw.tile([128, 128], bf16)
            w2_s = pw.tile([128, 2], bf16)
            ident = pw.tile([128, 128], bf16)
            ones2 = pw.tile([128, GROUP], bf16)
            iota_b = pw.tile([128, 128], bf16)
            affr8 = pw.tile([128, GROUP], bf16)
            epsc = pw.tile([128, 1], f32)
            b2c = pw.tile([2, 1], f32)
            ib_t = pw.tile([128, TB // 16], i16)
            sb_t = pw.tile([128, TB // 128], bf16)
            wb_t = pw.tile([128, TB // 128], bf16)
            nc.scalar.dma_start(w1_s[:], w1blk[:, :])
            nc.scalar.dma_start(b1_s[:], b1col[:, :])
            nc.scalar.dma_start(st_s[:], stats_lhsT[:, :])
            nc.scalar.dma_start(ga_s[:], gamma2[:, :])
            nc.scalar.dma_start(be_s[:], beta2[:, :])
            nc.scalar.dma_start(w2_s[:], w2col[:, :])
            nc.scalar.dma_start(ident[:], identd[:, :])
            nc.scalar.dma_start(iota_b[:], iota_d[:, :])
            nc.scalar.dma_start(ib_t[:], gidxB[:, :])
            nc.scalar.dma_start(sb_t[:], segB[:, :])
            nc.scalar.dma_start(wb_t[:], wgtB[:, :])
            nc.vector.memset(ones2[:], 1.0)
            nc.vector.memset(epsc[:], LN_EPS)
            nc.vector.memset(b2c[:], b2v)

            blk_chunks = {}
            for ck, (nb, prt) in enumerate(schedB):
                if nb >= 0:
                    blk_chunks.setdefault(nb, []).append((ck, prt))

            cemb_ap = bass.AP(cembg, 0, [[128, RC2], [1, 128]])
            SLAB = 4096
            slab_chunks = SLAB // 128
            gpc = GROUP // 128

            with tc.tile_pool(name="pm", bufs=2) as pm, \
                 tc.tile_pool(name="pBg", bufs=8) as pBg, \
                 tc.tile_pool(name="pp", bufs=2, space="PSUM") as pp, \
                 tc.tile_pool(name="pp1", bufs=1, space="PSUM") as pp1, \
                 tc.tile_pool(name="ppB", bufs=2, space="PSUM") as ppB:

                gtiles = {}

                def get_gtile(call):
                    if call not in gtiles:
                        t = pBg.tile([128, KC, 128], bf16, tag="gB",
                                     bufs=10)
                        nc.gpsimd.dma_gather(
                            t[:], cemb_ap,
                            ib_t[:, call * TG // 16:(call + 1) * TG // 16],
                            TG, TG, 128, queue_num=call % 4)
                        ind = pBg.tile([128, KC, 128], bf16, tag="indB",
                                       bufs=2)
                        nc.vector.tensor_tensor(
                            ind[:],
                            iota_b[:].unsqueeze(1)
                            .broadcast_to([128, KC, 128]),
                            (sb_t[:, call * KC:(call + 1) * KC]
                             .unsqueeze(2).broadcast_to([128, KC, 128])),
                            ALU.is_equal)
                        indw = pBg.tile([128, KC, 128], bf16, tag="indw",
                                        bufs=12)
                        nc.vector.tensor_mul(
                            indw[:], ind[:],
                            (wb_t[:, call * KC:(call + 1) * KC]
                             .unsqueeze(2).broadcast_to([128, KC, 128])))
                        gtiles[call] = (t, indw)
                    return gtiles[call]

                # last call needed by each block
                blk_last_call = {}
                for nb, cks in blk_chunks.items():
                    blk_last_call[nb] = max(ck for ck, _p in cks) // KC

                import os as _os
                KSTUB = _os.environ.get("KSTUB", "0") == "1"
                n_slab = (NTOK + SLAB - 1) // SLAB
                for s in range(n_slab):
                    t0 = s * SLAB
                    t1 = min(t0 + SLAB, NTOK)
                    ntok = t1 - t0
                    nch = ntok // 128
                    et = pm.tile([128, slab_chunks, H], bf16, tag="embm")
                    xt = pm.tile([128, slab_chunks, H], bf16, tag="xm")
                    lgslab = None
                    if not KSTUB:
                        lgslab = pm.tile([2, (SLAB // GROUP) * GROUP], f32,
                                         tag="lgs")
                    nc.scalar.dma_start(
                        et[:, :nch, :],
                        bass.AP(embN, t0 * H,
                                [[H, 128], [128 * H, nch], [1, H]]))

                    # issue gathers+indicators in groups of 8 calls, then
                    # process the blocks fully covered so far
                    c_lo = min(blk_last_call[t0 // 128 + j]
                               for j in range(nch))
                    c_hi = max(blk_last_call[t0 // 128 + j]
                               for j in range(nch))
                    jdone = 0
                    for cg in range(c_lo, c_hi + 1, 8):
                        for call in range(cg, min(cg + 8, c_hi + 1)):
                            get_gtile(call)
                        if KSTUB:
                            continue
                        cov = min(cg + 8, c_hi + 1) - 1
                        while jdone < nch and \
                                blk_last_call[t0 // 128 + jdone] <= cov:
                            j = jdone
                            nb = t0 // 128 + j
                            chunks = blk_chunks.get(nb, [])
                            ps = ppB.tile([128, 64], f32, tag="psB")
                            nc.tensor.matmul(ps[:], ident[:], et[:, j, :],
                                             start=True, stop=False)
                            nck = len(chunks)
                            for i, (ck, prt) in enumerate(chunks):
                                tl, ind_t = get_gtile(ck // KC)
                                nc.tensor.matmul(
                                    ps[:], ind_t[:, ck % KC, :],
                                    tl[:, ck % KC, 64 * prt:64 * prt + 64],
                                    start=False, stop=(i == nck - 1))
                            nc.scalar.copy(xt[:, j, :], ps[:])
                            jdone += 1

                    # ---- MLP: groups of 4 pairs share one stats chain
                    npr = 0 if KSTUB else ntok // (2 * GROUP)
                    for pg0 in range(0, npr, 3):
                        prs = list(range(pg0, min(pg0 + 3, npr)))
                        pst8 = pp1.tile([128, GROUP], f32, tag="pst8")
                        psq8 = pp1.tile([128, GROUP], f32, tag="psq8")
                        h1s = {}
                        for jl, pr in enumerate(prs):
                            xT = pp.tile([128, GROUP], bf16, tag="mmp")
                            for jj in range(gpc):
                                c0 = 2 * (pr * gpc + jj)
                                nc.tensor.transpose(
                                    xT[:, jj * 128:(jj + 1) * 128],
                                    xt[:, c0:c0 + 2, :], ident[:])
                            xT_sb = pm.tile([128, GROUP], bf16, tag="xTsb",
                                            bufs=3)
                            nc.scalar.copy(xT_sb[:], xT[:])
                            ph = pp.tile([128, GROUP], f32, tag="mmp")
                            nc.tensor.matmul(ph[:], w1_s[:], xT_sb[:])
                            h1 = pm.tile([128, GROUP], bf16, tag="h1",
                                         bufs=6)
                            sq = pm.tile([128, GROUP], bf16, tag="sq",
                                         bufs=3)
                            nc.vector.tensor_scalar(
                                h1[:], ph[:], b1_s[:], None, ALU.add,
                                ALU.bypass)
                            nc.vector.tensor_mul(sq[:], h1[:], h1[:])
                            nc.tensor.matmul(
                                pst8[32 * jl:32 * jl + 2, :], st_s[:], h1[:])
                            nc.tensor.matmul(
                                psq8[32 * jl:32 * jl + 2, :], st_s[:], sq[:])
                            h1s[pr] = h1
                        # stats lhsT carries 1/H: pst8 = mean, psq8 = E[h^2]
                        nrow = 32 * (len(prs) - 1) + 2
                        sm8 = pm.tile([128, GROUP], f32, tag="sm8",
                                      bufs=1)
                        var8 = pm.tile([128, GROUP], f32, tag="var8",
                                       bufs=1)
                        sd8 = pm.tile([128, GROUP], f32, tag="sd8", bufs=1)
                        rstd8 = pm.tile([128, GROUP], f32, tag="rstd8",
                                        bufs=1)
                        rstd8_bf = pm.tile([128, GROUP], bf16, tag="rstd8b",
                                           bufs=1)
                        nc.scalar.copy(sm8[:nrow, :], pst8[:nrow, :])
                        nc.vector.scalar_tensor_tensor(
                            var8[:nrow, :], sm8[:nrow, :], -1.0,
                            sm8[:nrow, :], ALU.mult, ALU.mult)
                        nc.vector.scalar_tensor_tensor(
                            var8[:nrow, :], psq8[:nrow, :], 1.0,
                            var8[:nrow, :], ALU.mult, ALU.add)
                        nc.scalar.activation(sd8[:nrow, :], var8[:nrow, :],
                                             AF.Sqrt, bias=epsc[:nrow, :],
                                             scale=1.0)
                        nc.vector.reciprocal_approx_fast(rstd8[:nrow, :],
                                                         sd8[:nrow, :])
                        nc.scalar.copy(rstd8_bf[:nrow, :], rstd8[:nrow, :])
                        nc.vector.scalar_tensor_tensor(
                            affr8[:nrow, :], sm8[:nrow, :], -1.0,
                            rstd8[:nrow, :], ALU.mult, ALU.mult)
                        for jl, pr in enumerate(prs):
                            g0 = t0 // GROUP + 2 * pr
                            h1 = h1s[pr]
                            pscale = pp1.tile([128, GROUP], f32,
                                              tag="pscale")
                            poff = pp1.tile([128, GROUP], f32, tag="poff")
                            sl = slice(32 * jl, 32 * jl + 2)
                            nc.tensor.matmul(
                                pscale[:], ga_s[sl, :], rstd8_bf[sl, :])
                            nc.tensor.matmul(
                                poff[:], ga_s[sl, :], affr8[sl, :],
                                start=True, stop=False)
                            nc.tensor.matmul(poff[:], be_s[sl, :],
                                             ones2[sl, :],
                                             start=False, stop=True)
                            t1t = pm.tile([128, GROUP], f32, tag="t1t")
                            h3 = pm.tile([128, GROUP], bf16, tag="h3")
                            nc.vector.tensor_mul(t1t[:], h1[:], pscale[:])
                            nc.vector.tensor_add(t1t[:], t1t[:], poff[:])
                            nc.scalar.activation(h3[:], t1t[:], AF.Relu)
                            pL2 = pp1.tile([2, GROUP], f32, tag="pst8")
                            nc.tensor.matmul(pL2[:], w2_s[:], h3[:])
                            nc.scalar.activation(
                                lgslab[:, pr * GROUP:(pr + 1) * GROUP],
                                pL2[:], AF.Identity, bias=b2c[:], scale=1.0)
                    if not KSTUB:
                        nc.scalar.dma_start(
                            bass.AP(out, (t0 // GROUP) * GROUP,
                                    [[GROUP, 2],
                                     [2 * GROUP, ntok // (2 * GROUP)],
                                     [1, GROUP]]),
                            lgslab[:, :ntok // 2])

    nc.compile()
    return nc


def weight_tensors(inputs):
    import ml_dtypes
    bf = ml_dtypes.bfloat16
    W1 = np.asarray(inputs["W1"], dtype=np.float32)
    b1 = np.asarray(inputs["b1"], dtype=np.float32)
    gamma = np.asarray(inputs["gamma"], dtype=np.float32)
    beta = np.asarray(inputs["beta"], dtype=np.float32)
    W2 = np.asarray(inputs["W2"], dtype=np.float32)
    w1blk = np.zeros((128, 128), dtype=np.float32)
    w1blk[:H, :H] = W1
    w1blk[H:, H:] = W1
    b1col = np.concatenate([b1, b1]).reshape(128, 1).astype(np.float32)
    stats = np.zeros((128, 2), dtype=np.float32)
    stats[:H, 0] = 1.0 / H
    stats[H:, 1] = 1.0 / H
    gamma2 = np.zeros((128, 128), dtype=np.float32)
    beta2 = np.zeros((128, 128), dtype=np.float32)
    for base in (0, 32, 64):
        gamma2[base, :H] = gamma
        gamma2[base + 1, H:] = gamma
        beta2[base, :H] = beta
        beta2[base + 1, H:] = beta
    w2col = np.zeros((128, 2), dtype=np.float32)
    w2col[:H, 0] = W2[:, 0]
    w2col[H:, 1] = W2[:, 0]
    iota = np.tile(np.arange(128, dtype=np.float32), (128, 1))
    return dict(
        w1blk=w1blk.astype(bf), b1col=b1col,
        stats_lhsT=stats.astype(bf), gamma2=gamma2.astype(bf),
        beta2=beta2.astype(bf), w2col=w2col.astype(bf),
        identd=np.eye(128, dtype=np.float32).astype(bf),
        iota_d=iota.astype(bf))


def build_in_maps(cfg, inputs, pre, wts, aux):
    import ml_dtypes
    bf = ml_dtypes.bfloat16
    emb_full = np.asarray(inputs["embedding"], dtype=np.float32)
    CH, CBH = cfg["CH"], cfg["CBH"]
    nodes_h, n_h = aux["nodes_h"], aux["n_h"]
    in_maps = []
    for core in range(cfg["n_cores"]):
        b, h = core // 2, core % 2
        d = pre[h]
        stream = emb_full[b][nodes_h[h]].astype(bf)     # [n_h, H] class-sorted
        embN_a = np.zeros((cfg["NTOK"], H), dtype=bf)
        embN_a[:n_h[h]] = stream
        embA_a = np.zeros((cfg["TA"], H), dtype=bf)
        valid = d["rowsrcA"] >= 0
        embA_a[valid] = stream[d["rowsrcA"][valid]]
        invc_h = aux["invc"][h * CH:(h + 1) * CH]
        invc_h = np.pad(invc_h, (0, CBH * 128 - len(invc_h)),
                        constant_values=1.0)
        m = dict(
            embA=embA_a, segA=d["segA"], embN=embN_a,
            gidxB=d["gidxB"], segB=d["segB"], wgtB=d["wgtB"],
            invc_tok=_cols128(invc_h), **wts)
        in_maps.append(m)
    return in_maps


def assemble_out(cfg, aux, results):
    B, N = cfg["B"], cfg["N"]
    nodes_h, n_h = aux["nodes_h"], aux["n_h"]
    npair = cfg["NGRP"] // 2
    out = np.empty((B, N), dtype=np.float32)
    for core in range(cfg["n_cores"]):
        b, h = core // 2, core % 2
        a = np.asarray(results[core]["out"]).reshape(npair, 2, 4, 128)
        o = a.transpose(0, 2, 1, 3).reshape(-1)[:n_h[h]]
        out[b, nodes_h[h]] = o
    return out


def kernel(**inputs):
    emb = np.asarray(inputs["embedding"])
    B, N, _ = emb.shape
    C = int(inputs["num_classes"])
    E = len(np.asarray(inputs["n2c_row"]))
    cfg = make_cfg(B, N, C, E)
    pre, meta, aux = host_prep(cfg, inputs)
    wts = weight_tensors(inputs)
    wvals = dict(b2=float(np.asarray(inputs["b2"]).reshape(-1)[0]))
    nc = build(cfg, meta, wvals)
    in_maps = build_in_maps(cfg, inputs, pre, wts, aux)
    from concourse.bass_utils import run_bass_kernel_spmd
    res = run_bass_kernel_spmd(nc, in_maps,
                               core_ids=list(range(cfg["n_cores"])))
    return assemble_out(cfg, aux, res.results)


# revision 18
# speedup vs baseline: 1.0247x; 1.0247x over previous
"""V3: class-half sharded GNN kernel, gather-free stage A.

Core (b, h) owns batch b and CLASS-half h: the nodes whose own class
(c2n_row) falls in half h.  Stage A needs no AllReduce: each core computes
complete class sums for its half from host-presorted contiguous bf16 rows
(class-block padded) via indicator matmuls, normalizes, and writes a bf16
paired-class half-table; a pairwise AllGather concatenates the halves.
Stage B gathers paired-class rows per edge (dma_gather, deep-buffered so
the ~2.2us/call cadence is not consumer-stalled) and indicator-matmuls
node-context blocks; fused LayerNorm MLP with Rsqrt activation.
"""

import numpy as np

SCR = 128
H = 64
LN_EPS = 1e-5


def _ru(x, m):
    return (x + m - 1) // m * m


def _wrap16(idx):
    n = len(idx)
    n16 = _ru(n, 16)
    a = np.full(n16, -1, dtype=np.int16)
    a[:n] = idx
    a = a.reshape(n16 // 16, 16).T
    return np.tile(a, (8, 1)).copy()


def _cols128(vals, dtype=np.float32):
    n = len(vals)
    return np.asarray(vals, dtype=dtype).reshape(n // 128, 128).T.copy()


def make_cfg(B, N, C, E, tg=1024):
    cfg = dict(B=B, N=N, C=C, E=E, n_cores=2 * B, TG=tg)
    cfg["GROUP"] = 512
    cfg["CPAD"] = _ru(C, 256)
    cfg["CH"] = cfg["CPAD"] // 2          # classes per half
    cfg["QH"] = cfg["CH"] // 2            # paired-class rows per half
    cfg["QT"] = 2 * cfg["QH"]
    cfg["CBH"] = cfg["CH"] // 128         # class blocks per half
    cfg["RC2"] = SCR + cfg["QT"] + SCR    # gather table rows
    assert SCR + cfg["QT"] < 32768
    return cfg


def host_prep(cfg, inputs):
    N, C, CH = cfg["N"], cfg["C"], cfg["CH"]
    TG, CBH = cfg["TG"], cfg["CBH"]
    c2n_row = np.asarray(inputs["c2n_row"]).astype(np.int64)
    n2c_row = np.asarray(inputs["n2c_row"]).astype(np.int64)
    n2c_col = np.asarray(inputs["n2c_col"]).astype(np.int64)

    cnt_c = np.bincount(c2n_row, minlength=C).astype(np.float32)
    invc = (1.0 / np.maximum(cnt_c, 1.0)).astype(np.float32)
    cnt_n = np.bincount(n2c_row, minlength=N).astype(np.float32)
    invn = (1.0 / np.maximum(cnt_n, 1.0)).astype(np.float32)

    # ---------- node streams per class-half
    nodes_h, order_h = {}, {}
    for h in (0, 1):
        sel = np.nonzero((c2n_row >= h * CH) & (c2n_row < (h + 1) * CH))[0]
        order = np.argsort(c2n_row[sel], kind="stable")
        nodes_h[h] = sel[order]           # original node ids, class-sorted
    n_h = {h: len(nodes_h[h]) for h in (0, 1)}
    NTOK = _ru(max(n_h.values()), 1024)
    cfg["NTOK"] = NTOK
    cfg["NGRP"] = NTOK // cfg["GROUP"]
    NBLK = NTOK // 128
    cfg["NBLK"] = NBLK

    # ---------- stage A caps (shared): members per class block
    capsA = np.zeros(CBH, dtype=np.int64)
    for h in (0, 1):
        cls_loc = c2n_row[nodes_h[h]] - h * CH
        capsA = np.maximum(capsA, np.bincount(cls_loc // 128, minlength=CBH))
    capsA = _ru(np.maximum(capsA, 1), 128)
    TA = int(capsA.sum())
    TA_pad = _ru(TA, TG)
    schedA = []
    for cb in range(CBH):
        for k in range(capsA[cb] // 128):
            schedA.append((cb, k == 0))
    for _ in range((TA_pad - TA) // 128):
        schedA.append((-1, False))
    cfg["TA"] = TA_pad

    # ---------- stage B caps: edges per (node block, parity)
    edges_h = {}
    capsB = np.zeros((NBLK, 2), dtype=np.int64)
    for h in (0, 1):
        pos = np.full(N, -1, dtype=np.int64)
        pos[nodes_h[h]] = np.arange(n_h[h])
        sel = np.nonzero(pos[n2c_row] >= 0)[0]
        dst = pos[n2c_row[sel]]
        col = n2c_col[sel]
        par = col % 2
        order = np.lexsort((par, dst // 128))
        edges_h[h] = (dst[order], col[order], par[order])
        nbk = dst[order] // 128
        for prt in (0, 1):
            cnt = np.bincount(nbk[par[order] == prt], minlength=NBLK)
            capsB[:, prt] = np.maximum(capsB[:, prt], cnt)
    capsB = _ru(np.maximum(capsB, 1), 128)
    TBn = int(capsB.sum())
    TB = _ru(TBn, TG)
    cfg["TB"] = TB
    schedB = []
    for nb in range(NBLK):
        for prt in (0, 1):
            for k in range(capsB[nb, prt] // 128):
                schedB.append((nb, prt))
    for _ in range((TB - TBn) // 128):
        schedB.append((-1, 0))

    # ---------- per-half index arrays
    pre = {}
    for h in (0, 1):
        d = {}
        # stage A: padded row placement + segids
        cls_loc = c2n_row[nodes_h[h]] - h * CH
        segA = np.full(cfg["TA"], 255, dtype=np.float32)
        rowsrc = np.full(cfg["TA"], -1, dtype=np.int64)  # index into node stream
        cnts = np.bincount(cls_loc // 128, minlength=CBH)
        starts = np.r_[0, np.cumsum(cnts)]
        base = 0
        for cb in range(CBH):
            nmem = int(cnts[cb])
            s = starts[cb]
            rowsrc[base:base + nmem] = np.arange(s, s + nmem)
            segA[base:base + nmem] = cls_loc[s:s + nmem] - 128 * cb
            base += capsA[cb]
        import ml_dtypes
        d["rowsrcA"] = rowsrc
        d["segA"] = _cols128(segA, ml_dtypes.bfloat16)

        # stage B
        dst, col, par = edges_h[h]
        nbk = dst // 128
        gidx = np.zeros(TB, dtype=np.int64)
        segB = np.full(TB, 255, dtype=np.float32)
        wgtB = np.zeros(TB, dtype=np.float32)
        base = 0
        for nb in range(NBLK):
            for prt in (0, 1):
                m = (nbk == nb) & (par == prt)
                nmem = int(m.sum())
                gidx[base:base + nmem] = SCR + col[m] // 2
                segB[base:base + nmem] = dst[m] - 128 * nb
                wgtB[base:base + nmem] = invn[nodes_h[h][dst[m]]]
                base += capsB[nb, prt]
        import ml_dtypes
        bf = ml_dtypes.bfloat16
        d["gidxB"] = _wrap16(gidx)
        d["segB"] = _cols128(segB, bf)
        d["wgtB"] = _cols128(wgtB, bf)
        pre[h] = d

    meta = dict(schedA=schedA, schedB=schedB)
    return pre, meta, dict(invc=invc, nodes_h=nodes_h, n_h=n_h)


def build(cfg, meta, wvals):
    from concourse import bass, bacc, tile, mybir

    f32 = mybir.dt.float32
    bf16 = mybir.dt.bfloat16
    i16 = mybir.dt.int16
    AF = mybir.ActivationFunctionType
    ALU = mybir.AluOpType

    CH, QH, QT = cfg["CH"], cfg["QH"], cfg["QT"]
    CBH, RC2 = cfg["CBH"], cfg["RC2"]
    NTOK, TG, GROUP = cfg["NTOK"], cfg["TG"], cfg["GROUP"]
    NBLK, TA, TB = cfg["NBLK"], cfg["TA"], cfg["TB"]
    n_cores = cfg["n_cores"]
    schedA, schedB = meta["schedA"], meta["schedB"]
    KC = TG // 128

    nc = bacc.Bacc("TRN2", target_bir_lowering=False, debug=False,
                   num_devices=n_cores, num_swdge_queues=4)

    embA = nc.dram_tensor("embA", [TA, H], bf16, kind="ExternalInput")
    segA = nc.dram_tensor("segA", [128, TA // 128], bf16, kind="ExternalInput")
    embN = nc.dram_tensor("embN", [NTOK, H], bf16, kind="ExternalInput")
    gidxB = nc.dram_tensor("gidxB", [128, TB // 16], i16, kind="ExternalInput")
    segB = nc.dram_tensor("segB", [128, TB // 128], bf16, kind="ExternalInput")
    wgtB = nc.dram_tensor("wgtB", [128, TB // 128], bf16, kind="ExternalInput")
    invc_tok = nc.dram_tensor("invc_tok", [128, CBH], f32,
                              kind="ExternalInput")
    iota_d = nc.dram_tensor("iota_d", [128, 128], bf16, kind="ExternalInput")
    w1blk = nc.dram_tensor("w1blk", [128, 128], bf16, kind="ExternalInput")
    b1col = nc.dram_tensor("b1col", [128, 1], f32, kind="ExternalInput")
    stats_lhsT = nc.dram_tensor("stats_lhsT", [128, 2], bf16,
                                kind="ExternalInput")
    gamma2 = nc.dram_tensor("gamma2", [128, 128], bf16, kind="ExternalInput")
    beta2 = nc.dram_tensor("beta2", [128, 128], bf16, kind="ExternalInput")
    w2col = nc.dram_tensor("w2col", [128, 2], bf16, kind="ExternalInput")
    identd = nc.dram_tensor("identd", [128, 128], bf16, kind="ExternalInput")
    out = nc.dram_tensor("out", [cfg["NGRP"], GROUP], f32,
                         kind="ExternalOutput")

    chalf = nc.dram_tensor("chalf", [QH, 128], bf16)
    cemb2 = nc.dram_tensor("cemb2", [RC2, 128], bf16)
    cembg = nc.dram_tensor("cembg", [RC2, 128], bf16)

    b2v = float(wvals["b2"])

    with tile.TileContext(nc) as tc:
        # ================= stage A =================
        with tc.tile_pool(name="pA", bufs=1) as pA, \
             tc.tile_pool(name="pAg", bufs=4) as pAg, \
             tc.tile_pool(name="pAn", bufs=3) as pAn, \
             tc.tile_pool(name="pAp", bufs=4, space="PSUM") as pAp:
            csum_sb = pA.tile([128, CBH, H], f32)
            sa = pA.tile([128, TA // 128], bf16)
            iota_s = pA.tile([128, 128], bf16)
            iv = pA.tile([128, CBH], f32)
            zb = pA.tile([128, SCR], bf16)
            nc.scalar.dma_start(sa[:], segA[:, :])
            nc.scalar.dma_start(iota_s[:], iota_d[:, :])
            nc.scalar.dma_start(iv[:], invc_tok[:, :])
            nc.vector.memset(csum_sb[:], 0.0)
            nc.vector.memset(zb[:], 0.0)
            # zero guard rows of the gather table
            nc.scalar.dma_start(bass.AP(cemb2, 0, [[128, SCR], [1, 128]]),
                                zb[:])

            cur_tile = None
            cur_ind = None
            pend = []
            prev_cb = None

            def flushA(pend, cb):
                if not pend:
                    return
                ps = pAp.tile([128, H], f32, tag="psA")
                for i, (tl, ind_t, kk) in enumerate(pend):
                    nc.tensor.matmul(ps[:], ind_t[:, kk, :], tl[:, kk, :],
                                     start=(i == 0),
                                     stop=(i == len(pend) - 1))
                nc.vector.tensor_add(csum_sb[:, cb, :],
                                     csum_sb[:, cb, :], ps[:])

            for ck, (cb, first) in enumerate(schedA):
                call = ck // KC
                if ck % KC == 0:
                    cur_tile = pAg.tile([128, KC, H], bf16, tag="gA")
                    nc.scalar.dma_start(
                        cur_tile[:],
                        bass.AP(embA, call * TG * H,
                                [[H, 128], [128 * H, KC], [1, H]]))
                    cur_ind = pAg.tile([128, KC, 128], bf16, tag="indA")
                    nc.vector.tensor_tensor(
                        cur_ind[:],
                        iota_s[:].unsqueeze(1).broadcast_to([128, KC, 128]),
                        (sa[:, call * KC:(call + 1) * KC]
                         .unsqueeze(2).broadcast_to([128, KC, 128])),
                        ALU.is_equal)
                if cb < 0:
                    continue
                if cb != prev_cb:
                    flushA(pend, prev_cb)
                    pend = []
                    prev_cb = cb
                pend.append((cur_tile, cur_ind, ck % KC))
            flushA(pend, prev_cb)

            # normalize -> bf16 paired half table (class c at elems c*64)
            NBS = 32   # blocks per normalize slab
            cb0 = 0
            while cb0 < CBH:
                nb_ = min(NBS, CBH - cb0)
                tbf = pAn.tile([128, NBS, H], bf16, tag="nrmb")
                ivb = (iv[:, cb0:cb0 + nb_]
                       .unsqueeze(2).broadcast_to([128, nb_, H]))
                nc.vector.tensor_mul(tbf[:, :nb_, :],
                                     csum_sb[:, cb0:cb0 + nb_, :], ivb)
                nc.scalar.dma_start(
                    bass.AP(chalf, cb0 * 128 * H,
                            [[H, 128], [128 * H, nb_], [1, H]]),
                    tbf[:, :nb_, :])
                cb0 += nb_

        # ================= AllGather halves =================
        groups = [[2 * b, 2 * b + 1] for b in range(n_cores // 2)]
        nc.gpsimd.collective_compute(
            "AllGather", mybir.AluOpType.bypass,
            replica_groups=groups,
            ins=[bass.AP(chalf, 0, [[128, QH], [1, 128]]).opt()],
            outs=[bass.AP(cemb2, SCR * 128, [[128, QT], [1, 128]]).opt()])

        # copy AG output out of the collectives pool (random reads there
        # are ~4x slower) into a plain internal tensor for the gathers
        with tc.tile_pool(name="pcp", bufs=2) as pcp:
            CPR = 4224   # rows per copy slab; RC2 = 198*128
            r0 = 0
            while r0 < RC2:
                rr = min(CPR, RC2 - r0)
                nb_ = rr // 128
                tcp = pcp.tile([128, CPR // 128, 128], bf16, tag="cp")
                nc.scalar.dma_start(
                    tcp[:, :nb_, :],
                    bass.AP(cemb2, r0 * 128,
                            [[128, 128], [128 * 128, nb_], [1, 128]]))
                nc.scalar.dma_start(
                    bass.AP(cembg, r0 * 128,
                            [[128, 128], [128 * 128, nb_], [1, 128]]),
                    tcp[:, :nb_, :])
                r0 += rr

        # ================= stage B + MLP =================
        with tc.tile_pool(name="pw", bufs=1) as pw:
            w1_s = pw.tile([128, 128], bf16)
            b1_s = pw.tile([128, 1], f32)
            st_s = pw.tile([128, 2], bf16)
            ga_s = pw.tile([128, 128], bf16)
            be_s = pw.tile([128, 128], bf16)
            w2_s = pw.tile([128, 2], bf16)
            ident = pw.tile([128, 128], bf16)
            ones2 = pw.tile([128, GROUP], bf16)
            iota_b = pw.tile([128, 128], bf16)
            affr8 = pw.tile([128, GROUP], bf16)
            epsc = pw.tile([128, 1], f32)
            b2c = pw.tile([2, 1], f32)
            ib_t = pw.tile([128, TB // 16], i16)
            sb_t = pw.tile([128, TB // 128], bf16)
            wb_t = pw.tile([128, TB // 128], bf16)
            nc.scalar.dma_start(w1_s[:], w1blk[:, :])
            nc.scalar.dma_start(b1_s[:], b1col[:, :])
            nc.scalar.dma_start(st_s[:], stats_lhsT[:, :])
            nc.scalar.dma_start(ga_s[:], gamma2[:, :])
            nc.scalar.dma_start(be_s[:], beta2[:, :])
            nc.scalar.dma_start(w2_s[:], w2col[:, :])
            nc.scalar.dma_start(ident[:], identd[:, :])
            nc.scalar.dma_start(iota_b[:], iota_d[:, :])
            nc.scalar.dma_start(ib_t[:], gidxB[:, :])
            nc.scalar.dma_start(sb_t[:], segB[:, :])
            nc.scalar.dma_start(wb_t[:], wgtB[:, :])
            nc.vector.memset(ones2[:], 1.0)
            nc.vector.memset(epsc[:], LN_EPS)
            nc.vector.memset(b2c[:], b2v)

            blk_chunks = {}
            for ck, (nb, prt) in enumerate(schedB):
                if nb >= 0:
                    blk_chunks.setdefault(nb, []).append((ck, prt))

            cemb_ap = bass.AP(cembg, 0, [[128, RC2], [1, 128]])
            SLAB = 4096
            slab_chunks = SLAB // 128
            gpc = GROUP // 128

            with tc.tile_pool(name="pm", bufs=2) as pm, \
                 tc.tile_pool(name="pBg", bufs=8) as pBg, \
                 tc.tile_pool(name="pp", bufs=2, space="PSUM") as pp, \
                 tc.tile_pool(name="pp1", bufs=1, space="PSUM") as pp1, \
                 tc.tile_pool(name="ppB", bufs=2, space="PSUM") as ppB:

                gtiles = {}

                def get_gtile(call):
                    if call not in gtiles:
                        t = pBg.tile([128, KC, 128], bf16, tag="gB",
                                     bufs=10)
                        nc.gpsimd.dma_gather(
                            t[:], cemb_ap,
                            ib_t[:, call * TG // 16:(call + 1) * TG // 16],
                            TG, TG, 128, queue_num=call % 4)
                        ind = pBg.tile([128, KC, 128], bf16, tag="indB",
                                       bufs=2)
                        nc.vector.tensor_tensor(
                            ind[:],
                            iota_b[:].unsqueeze(1)
                            .broadcast_to([128, KC, 128]),
                            (sb_t[:, call * KC:(call + 1) * KC]
                             .unsqueeze(2).broadcast_to([128, KC, 128])),
                            ALU.is_equal)
                        indw = pBg.tile([128, KC, 128], bf16, tag="indw",
                                        bufs=12)
                        nc.vector.tensor_mul(
                            indw[:], ind[:],
                            (wb_t[:, call * KC:(call + 1) * KC]
                             .unsqueeze(2).broadcast_to([128, KC, 128])))
                        gtiles[call] = (t, indw)
                    return gtiles[call]

                # last call needed by each block
                blk_last_call = {}
                for nb, cks in blk_chunks.items():
                    blk_last_call[nb] = max(ck for ck, _p in cks) // KC

                import os as _os
                KSTUB = _os.environ.get("KSTUB", "0") == "1"
                n_slab = (NTOK + SLAB - 1) // SLAB
                for s in range(n_slab):
                    t0 = s * SLAB
                    t1 = min(t0 + SLAB, NTOK)
                    ntok = t1 - t0
                    nch = ntok // 128
                    et = pm.tile([128, slab_chunks, H], bf16, tag="embm")
                    xt = pm.tile([128, slab_chunks, H], bf16, tag="xm")
                    lgslab = None
                    if not KSTUB:
                        lgslab = pm.tile([2, (SLAB // GROUP) * GROUP], f32,
                                         tag="lgs")
                    nc.scalar.dma_start(
                        et[:, :nch, :],
                        bass.AP(embN, t0 * H,
                                [[H, 128], [128 * H, nch], [1, H]]))

                    # issue gathers+indicators in groups of 8 calls, then
                    # process the blocks fully covered so far
                    c_lo = min(blk_last_call[t0 // 128 + j]
                               for j in range(nch))
                    c_hi = max(blk_last_call[t0 // 128 + j]
                               for j in range(nch))
                    jdone = 0
                    for cg in range(c_lo, c_hi + 1, 8):
                        for call in range(cg, min(cg + 8, c_hi + 1)):
                            get_gtile(call)
                        if KSTUB:
                            continue
                        cov = min(cg + 8, c_hi + 1) - 1
                        while jdone < nch and \
                                blk_last_call[t0 // 128 + jdone] <= cov:
                            j = jdone
                            nb = t0 // 128 + j
                            chunks = blk_chunks.get(nb, [])
                            ps = ppB.tile([128, 64], f32, tag="psB")
                            nc.tensor.matmul(ps[:], ident[:], et[:, j, :],
                                             start=True, stop=False)
                            nck = len(chunks)
                            for i, (ck, prt) in enumerate(chunks):
                                tl, ind_t = get_gtile(ck // KC)
                                nc.tensor.matmul(
                                    ps[:], ind_t[:, ck % KC, :],
                                    tl[:, ck % KC, 64 * prt:64 * prt + 64],
                                    start=False, stop=(i == nck - 1))
                            nc.scalar.copy(xt[:, j, :], ps[:])
                            jdone += 1

                    # ---- MLP: groups of 4 pairs share one stats chain
                    npr = 0 if KSTUB else ntok // (2 * GROUP)
                    for pg0 in range(0, npr, 3):
                        prs = list(range(pg0, min(pg0 + 3, npr)))
                        pst8 = pp1.tile([128, GROUP], f32, tag="pst8")
                        psq8 = pp1.tile([128, GROUP], f32, tag="psq8")
                        h1s = {}
                        for jl, pr in enumerate(prs):
                            xT = pp.tile([128, GROUP], bf16, tag="mmp")
                            for jj in range(gpc):
                                c0 = 2 * (pr * gpc + jj)
                                nc.tensor.transpose(
                                    xT[:, jj * 128:(jj + 1) * 128],
                                    xt[:, c0:c0 + 2, :], ident[:])
                            xT_sb = pm.tile([128, GROUP], bf16, tag="xTsb",
                                            bufs=3)
                            nc.scalar.copy(xT_sb[:], xT[:])
                            ph = pp.tile([128, GROUP], f32, tag="mmp")
                            nc.tensor.matmul(ph[:], w1_s[:], xT_sb[:])
                            h1 = pm.tile([128, GROUP], bf16, tag="h1",
                                         bufs=6)
                            sq = pm.tile([128, GROUP], bf16, tag="sq",
                                         bufs=3)
                            nc.vector.tensor_scalar(
                                h1[:], ph[:], b1_s[:], None, ALU.add,
                                ALU.bypass)
                            nc.vector.tensor_mul(sq[:], h1[:], h1[:])
                            nc.tensor.matmul(
                                pst8[32 * jl:32 * jl + 2, :], st_s[:], h1[:])
                            nc.tensor.matmul(
                                psq8[32 * jl:32 * jl + 2, :], st_s[:], sq[:])
                            h1s[pr] = h1
                        # stats lhsT carries 1/H: pst8 = mean, psq8 = E[h^2]
                        nrow = 32 * (len(prs) - 1) + 2
                        sm8 = pm.tile([128, GROUP], f32, tag="sm8",
                                      bufs=1)
                        var8 = pm.tile([128, GROUP], f32, tag="var8",
                                       bufs=1)
                        sd8 = pm.tile([128, GROUP], f32, tag="sd8", bufs=1)
                        rstd8 = pm.tile([128, GROUP], f32, tag="rstd8",
                                        bufs=1)
                        rstd8_bf = pm.tile([128, GROUP], bf16, tag="rstd8b",
                                           bufs=1)
                        nc.scalar.copy(sm8[:nrow, :], pst8[:nrow, :])
                        nc.vector.scalar_tensor_tensor(
                            var8[:nrow, :], sm8[:nrow, :], -1.0,
                            sm8[:nrow, :], ALU.mult, ALU.mult)
                        nc.vector.scalar_tensor_tensor(
                            var8[:nrow, :], psq8[:nrow, :], 1.0,
                            var8[:nrow, :], ALU.mult, ALU.add)
                        nc.scalar.activation(sd8[:nrow, :], var8[:nrow, :],
                                             AF.Sqrt, bias=epsc[:nrow, :],
                                             scale=1.0)
                        nc.vector.reciprocal_approx_fast(rstd8[:nrow, :],
                                                         sd8[:nrow, :])
                        nc.scalar.copy(rstd8_bf[:nrow, :], rstd8[:nrow, :])
                        nc.vector.scalar_tensor_tensor(
                            affr8[:nrow, :], sm8[:nrow, :], -1.0,
                            rstd8[:nrow, :], ALU.mult, ALU.mult)
                        for jl, pr in enumerate(prs):
                            g0 = t0 // GROUP + 2 * pr
                            h1 = h1s[pr]
                            pscale = pp1.tile([128, GROUP], f32,
                                              tag="pscale")
                            poff = pp1.tile([128, GROUP], f32, tag="poff")
                            sl = slice(32 * jl, 32 * jl + 2)
                            nc.tensor.matmul(
                                pscale[:], ga_s[sl, :], rstd8_bf[sl, :])
                            nc.tensor.matmul(
                                poff[:], ga_s[sl, :], affr8[sl, :],
                                start=True, stop=False)
                            nc.tensor.matmul(poff[:], be_s[sl, :],
                                             ones2[sl, :],
                                             start=False, stop=True)
                            t1t = pm.tile([128, GROUP], f32, tag="t1t")
                            h3 = pm.tile([128, GROUP], bf16, tag="h3")
                            nc.vector.tensor_mul(t1t[:], h1[:], pscale[:])
                            nc.vector.tensor_add(t1t[:], t1t[:], poff[:])
                            nc.scalar.activation(h3[:], t1t[:], AF.Relu)
                            pL2 = pp1.tile([2, GROUP], f32, tag="pst8")
                            nc.tensor.matmul(pL2[:], w2_s[:], h3[:])
                            nc.scalar.activation(
                                lgslab[:, pr * GROUP:(pr + 1) * GROUP],
                                pL2[:], AF.Identity, bias=b2c[:], scale=1.0)
                    if not KSTUB:
                        nc.scalar.dma_start(
                            bass.AP(out, (t0 // GROUP) * GROUP,
                                    [[GROUP, 2],
                                     [2 * GROUP, ntok // (2 * GROUP)],
                                     [1, GROUP]]),
                            lgslab[:, :ntok // 2])

    nc.compile()
    return nc


def weight_tensors(inputs):
    import ml_dtypes
    bf = ml_dtypes.bfloat16
    W1 = np.asarray(inputs["W1"], dtype=np.float32)
    b1 = np.asarray(inputs["b1"], dtype=np.float32)
    gamma = np.asarray(inputs["gamma"], dtype=np.float32)
    beta = np.asarray(inputs["beta"], dtype=np.float32)
    W2 = np.asarray(inputs["W2"], dtype=np.float32)
    w1blk = np.zeros((128, 128), dtype=np.float32)
    w1blk[:H, :H] = W1
    w1blk[H:, H:] = W1
    b1col = np.concatenate([b1, b1]).reshape(128, 1).astype(np.float32)
    stats = np.zeros((128, 2), dtype=np.float32)
    stats[:H, 0] = 1.0 / H
    stats[H:, 1] = 1.0 / H
    gamma2 = np.zeros((128, 128), dtype=np.float32)
    beta2 = np.zeros((128, 128), dtype=np.float32)
    for base in (0, 32, 64):
        gamma2[base, :H] = gamma
        gamma2[base + 1, H:] = gamma
        beta2[base, :H] = beta
        beta2[base + 1, H:] = beta
    w2col = np.zeros((128, 2), dtype=np.float32)
    w2col[:H, 0] = W2[:, 0]
    w2col[H:, 1] = W2[:, 0]
    iota = np.tile(np.arange(128, dtype=np.float32), (128, 1))
    return dict(
        w1blk=w1blk.astype(bf), b1col=b1col,
        stats_lhsT=stats.astype(bf), gamma2=gamma2.astype(bf),
        beta2=beta2.astype(bf), w2col=w2col.astype(bf),
        identd=np.eye(128, dtype=np.float32).astype(bf),
        iota_d=iota.astype(bf))


def build_in_maps(cfg, inputs, pre, wts, aux):
    import ml_dtypes
    bf = ml_dtypes.bfloat16
    emb_full = np.asarray(inputs["embedding"], dtype=np.float32)
    CH, CBH = cfg["CH"], cfg["CBH"]
    nodes_h, n_h = aux["nodes_h"], aux["n_h"]
    in_maps = []
    for core in range(cfg["n_cores"]):
        b, h = core // 2, core % 2
        d = pre[h]
        stream = emb_full[b][nodes_h[h]].astype(bf)     # [n_h, H] class-sorted
        embN_a = np.zeros((cfg["NTOK"], H), dtype=bf)
        embN_a[:n_h[h]] = stream
        embA_a = np.zeros((cfg["TA"], H), dtype=bf)
        valid = d["rowsrcA"] >= 0
        embA_a[valid] = stream[d["rowsrcA"][valid]]
        invc_h = aux["invc"][h * CH:(h + 1) * CH]
        invc_h = np.pad(invc_h, (0, CBH * 128 - len(invc_h)),
                        constant_values=1.0)
        m = dict(
            embA=embA_a, segA=d["segA"], embN=embN_a,
            gidxB=d["gidxB"], segB=d["segB"], wgtB=d["wgtB"],
            invc_tok=_cols128(invc_h), **wts)
        in_maps.append(m)
    return in_maps


def assemble_out(cfg, aux, results):
    B, N = cfg["B"], cfg["N"]
    nodes_h, n_h = aux["nodes_h"], aux["n_h"]
    npair = cfg["NGRP"] // 2
    out = np.empty((B, N), dtype=np.float32)
    for core in range(cfg["n_cores"]):
        b, h = core // 2, core % 2
        a = np.asarray(results[core]["out"]).reshape(npair, 2, 4, 128)
        o = a.transpose(0, 2, 1, 3).reshape(-1)[:n_h[h]]
        out[b, nodes_h[h]] = o
    return out


def kernel(**inputs):
    emb = np.asarray(inputs["embedding"])
    B, N, _ = emb.shape
    C = int(inputs["num_classes"])
    E = len(np.asarray(inputs["n2c_row"]))
    cfg = make_cfg(B, N, C, E)
    pre, meta, aux = host_prep(cfg, inputs)
    wts = weight_tensors(inputs)
    wvals = dict(b2=float(np.asarray(inputs["b2"]).reshape(-1)[0]))
    nc = build(cfg, meta, wvals)
    in_maps = build_in_maps(cfg, inputs, pre, wts, aux)
    from concourse.bass_utils import run_bass_kernel_spmd
    res = run_bass_kernel_spmd(nc, in_maps,
                               core_ids=list(range(cfg["n_cores"])))
    return assemble_out(cfg, aux, res.results)


# revision 19
# speedup vs baseline: 2.2986x; 2.2432x over previous
"""V3: class-half sharded GNN kernel, gather-free stage A.

Core (b, h) owns batch b and CLASS-half h: the nodes whose own class
(c2n_row) falls in half h.  Stage A needs no AllReduce: each core computes
complete class sums for its half from host-presorted contiguous bf16 rows
(class-block padded) via indicator matmuls, normalizes, and writes a bf16
paired-class half-table; a pairwise AllGather concatenates the halves.
Stage B gathers paired-class rows per edge (dma_gather, deep-buffered so
the ~2.2us/call cadence is not consumer-stalled) and indicator-matmuls
node-context blocks; fused LayerNorm MLP with Rsqrt activation.
"""

import numpy as np

SCR = 128
H = 64
LN_EPS = 1e-5


def _ru(x, m):
    return (x + m - 1) // m * m


def _wrap16(idx):
    n = len(idx)
    n16 = _ru(n, 16)
    a = np.full(n16, -1, dtype=np.int16)
    a[:n] = idx
    a = a.reshape(n16 // 16, 16).T
    return np.tile(a, (8, 1)).copy()


def _cols128(vals, dtype=np.float32):
    n = len(vals)
    return np.asarray(vals, dtype=dtype).reshape(n // 128, 128).T.copy()


def make_cfg(B, N, C, E, tg=1024):
    cfg = dict(B=B, N=N, C=C, E=E, n_cores=2 * B, TG=tg)
    cfg["GROUP"] = 512
    cfg["CPAD"] = _ru(C, 256)
    cfg["CH"] = cfg["CPAD"] // 2          # classes per half
    cfg["QH"] = cfg["CH"] // 2            # paired-class rows per half
    cfg["QT"] = 2 * cfg["QH"]
    cfg["CBH"] = cfg["CH"] // 128         # class blocks per half
    cfg["RC2"] = SCR + cfg["QT"] + SCR    # gather table rows
    assert SCR + cfg["QT"] < 32768
    return cfg


def host_prep(cfg, inputs):
    N, C, CH = cfg["N"], cfg["C"], cfg["CH"]
    TG, CBH = cfg["TG"], cfg["CBH"]
    c2n_row = np.asarray(inputs["c2n_row"]).astype(np.int64)
    n2c_row = np.asarray(inputs["n2c_row"]).astype(np.int64)
    n2c_col = np.asarray(inputs["n2c_col"]).astype(np.int64)

    cnt_c = np.bincount(c2n_row, minlength=C).astype(np.float32)
    invc = (1.0 / np.maximum(cnt_c, 1.0)).astype(np.float32)
    cnt_n = np.bincount(n2c_row, minlength=N).astype(np.float32)
    invn = (1.0 / np.maximum(cnt_n, 1.0)).astype(np.float32)

    # ---------- node streams per class-half
    nodes_h, order_h = {}, {}
    for h in (0, 1):
        sel = np.nonzero((c2n_row >= h * CH) & (c2n_row < (h + 1) * CH))[0]
        order = np.argsort(c2n_row[sel], kind="stable")
        nodes_h[h] = sel[order]           # original node ids, class-sorted
    n_h = {h: len(nodes_h[h]) for h in (0, 1)}
    NTOK = _ru(max(n_h.values()), 1024)
    cfg["NTOK"] = NTOK
    cfg["NGRP"] = NTOK // cfg["GROUP"]
    NBLK = NTOK // 128
    cfg["NBLK"] = NBLK

    # ---------- stage A caps (shared): members per class block
    capsA = np.zeros(CBH, dtype=np.int64)
    for h in (0, 1):
        cls_loc = c2n_row[nodes_h[h]] - h * CH
        capsA = np.maximum(capsA, np.bincount(cls_loc // 128, minlength=CBH))
    capsA = _ru(np.maximum(capsA, 1), 128)
    TA = int(capsA.sum())
    TA_pad = _ru(TA, TG)
    schedA = []
    for cb in range(CBH):
        for k in range(capsA[cb] // 128):
            schedA.append((cb, k == 0))
    for _ in range((TA_pad - TA) // 128):
        schedA.append((-1, False))
    cfg["TA"] = TA_pad

    # ---------- stage B caps: edges per (node block, parity)
    edges_h = {}
    capsB = np.zeros((NBLK, 2), dtype=np.int64)
    for h in (0, 1):
        pos = np.full(N, -1, dtype=np.int64)
        pos[nodes_h[h]] = np.arange(n_h[h])
        sel = np.nonzero(pos[n2c_row] >= 0)[0]
        dst = pos[n2c_row[sel]]
        col = n2c_col[sel]
        par = col % 2
        order = np.lexsort((par, dst // 128))
        edges_h[h] = (dst[order], col[order], par[order])
        nbk = dst[order] // 128
        for prt in (0, 1):
            cnt = np.bincount(nbk[par[order] == prt], minlength=NBLK)
            capsB[:, prt] = np.maximum(capsB[:, prt], cnt)
    capsB = _ru(np.maximum(capsB, 1), 128)
    TBn = int(capsB.sum())
    TB = _ru(TBn, TG)
    cfg["TB"] = TB
    schedB = []
    for nb in range(NBLK):
        for prt in (0, 1):
            for k in range(capsB[nb, prt] // 128):
                schedB.append((nb, prt))
    for _ in range((TB - TBn) // 128):
        schedB.append((-1, 0))

    # ---------- per-half index arrays
    pre = {}
    for h in (0, 1):
        d = {}
        # stage A: padded row placement + segids
        cls_loc = c2n_row[nodes_h[h]] - h * CH
        segA = np.full(cfg["TA"], 255, dtype=np.float32)
        rowsrc = np.full(cfg["TA"], -1, dtype=np.int64)  # index into node stream
        cnts = np.bincount(cls_loc // 128, minlength=CBH)
        starts = np.r_[0, np.cumsum(cnts)]
        base = 0
        for cb in range(CBH):
            nmem = int(cnts[cb])
            s = starts[cb]
            rowsrc[base:base + nmem] = np.arange(s, s + nmem)
            segA[base:base + nmem] = cls_loc[s:s + nmem] - 128 * cb
            base += capsA[cb]
        import ml_dtypes
        d["rowsrcA"] = rowsrc
        d["segA"] = _cols128(segA, ml_dtypes.bfloat16)

        # stage B
        dst, col, par = edges_h[h]
        nbk = dst // 128
        # pad slots gather arbitrary (ignored) rows; spread them to avoid
        # all pads hammering one HBM row
        gidx = SCR + (np.arange(TB, dtype=np.int64) * 97) % cfg["QT"]
        segB = np.full(TB, 255, dtype=np.float32)
        wgtB = np.zeros(TB, dtype=np.float32)
        base = 0
        for nb in range(NBLK):
            for prt in (0, 1):
                m = (nbk == nb) & (par == prt)
                nmem = int(m.sum())
                gidx[base:base + nmem] = SCR + col[m] // 2
                segB[base:base + nmem] = dst[m] - 128 * nb
                wgtB[base:base + nmem] = invn[nodes_h[h][dst[m]]]
                base += capsB[nb, prt]
        import ml_dtypes
        bf = ml_dtypes.bfloat16
        d["gidxB"] = _wrap16(gidx)
        d["segB"] = _cols128(segB, bf)
        d["wgtB"] = _cols128(wgtB, bf)
        pre[h] = d

    meta = dict(schedA=schedA, schedB=schedB)
    return pre, meta, dict(invc=invc, nodes_h=nodes_h, n_h=n_h)


def build(cfg, meta, wvals):
    from concourse import bass, bacc, tile, mybir

    f32 = mybir.dt.float32
    bf16 = mybir.dt.bfloat16
    i16 = mybir.dt.int16
    AF = mybir.ActivationFunctionType
    ALU = mybir.AluOpType

    CH, QH, QT = cfg["CH"], cfg["QH"], cfg["QT"]
    CBH, RC2 = cfg["CBH"], cfg["RC2"]
    NTOK, TG, GROUP = cfg["NTOK"], cfg["TG"], cfg["GROUP"]
    NBLK, TA, TB = cfg["NBLK"], cfg["TA"], cfg["TB"]
    n_cores = cfg["n_cores"]
    schedA, schedB = meta["schedA"], meta["schedB"]
    KC = TG // 128

    nc = bacc.Bacc("TRN2", target_bir_lowering=False, debug=False,
                   num_devices=n_cores, num_swdge_queues=4)

    embA = nc.dram_tensor("embA", [TA, H], bf16, kind="ExternalInput")
    segA = nc.dram_tensor("segA", [128, TA // 128], bf16, kind="ExternalInput")
    embN = nc.dram_tensor("embN", [NTOK, H], bf16, kind="ExternalInput")
    gidxB = nc.dram_tensor("gidxB", [128, TB // 16], i16, kind="ExternalInput")
    segB = nc.dram_tensor("segB", [128, TB // 128], bf16, kind="ExternalInput")
    wgtB = nc.dram_tensor("wgtB", [128, TB // 128], bf16, kind="ExternalInput")
    invc_tok = nc.dram_tensor("invc_tok", [128, CBH], f32,
                              kind="ExternalInput")
    iota_d = nc.dram_tensor("iota_d", [128, 128], bf16, kind="ExternalInput")
    w1blk = nc.dram_tensor("w1blk", [128, 128], bf16, kind="ExternalInput")
    b1col = nc.dram_tensor("b1col", [128, 1], f32, kind="ExternalInput")
    stats_lhsT = nc.dram_tensor("stats_lhsT", [128, 2], bf16,
                                kind="ExternalInput")
    gamma2 = nc.dram_tensor("gamma2", [128, 128], bf16, kind="ExternalInput")
    beta2 = nc.dram_tensor("beta2", [128, 128], bf16, kind="ExternalInput")
    w2col = nc.dram_tensor("w2col", [128, 2], bf16, kind="ExternalInput")
    identd = nc.dram_tensor("identd", [128, 128], bf16, kind="ExternalInput")
    out = nc.dram_tensor("out", [cfg["NGRP"], GROUP], f32,
                         kind="ExternalOutput")

    chalf = nc.dram_tensor("chalf", [QH, 128], bf16)
    cemb2 = nc.dram_tensor("cemb2", [RC2, 128], bf16)
    cembg = nc.dram_tensor("cembg", [RC2, 128], bf16)

    b2v = float(wvals["b2"])

    with tile.TileContext(nc) as tc:
        # ================= stage A =================
        with tc.tile_pool(name="pA", bufs=1) as pA, \
             tc.tile_pool(name="pAg", bufs=4) as pAg, \
             tc.tile_pool(name="pAn", bufs=3) as pAn, \
             tc.tile_pool(name="pAp", bufs=4, space="PSUM") as pAp:
            csum_sb = pA.tile([128, CBH, H], f32)
            sa = pA.tile([128, TA // 128], bf16)
            iota_s = pA.tile([128, 128], bf16)
            iv = pA.tile([128, CBH], f32)
            zb = pA.tile([128, SCR], bf16)
            nc.scalar.dma_start(sa[:], segA[:, :])
            nc.scalar.dma_start(iota_s[:], iota_d[:, :])
            nc.scalar.dma_start(iv[:], invc_tok[:, :])
            nc.vector.memset(csum_sb[:], 0.0)
            nc.vector.memset(zb[:], 0.0)
            # zero guard rows of the gather table
            nc.scalar.dma_start(bass.AP(cemb2, 0, [[128, SCR], [1, 128]]),
                                zb[:])

            cur_tile = None
            cur_ind = None
            pend = []
            prev_cb = None

            def flushA(pend, cb):
                if not pend:
                    return
                ps = pAp.tile([128, H], f32, tag="psA")
                for i, (tl, ind_t, kk) in enumerate(pend):
                    nc.tensor.matmul(ps[:], ind_t[:, kk, :], tl[:, kk, :],
                                     start=(i == 0),
                                     stop=(i == len(pend) - 1))
                nc.vector.tensor_add(csum_sb[:, cb, :],
                                     csum_sb[:, cb, :], ps[:])

            for ck, (cb, first) in enumerate(schedA):
                call = ck // KC
                if ck % KC == 0:
                    cur_tile = pAg.tile([128, KC, H], bf16, tag="gA")
                    nc.scalar.dma_start(
                        cur_tile[:],
                        bass.AP(embA, call * TG * H,
                                [[H, 128], [128 * H, KC], [1, H]]))
                    cur_ind = pAg.tile([128, KC, 128], bf16, tag="indA")
                    nc.vector.tensor_tensor(
                        cur_ind[:],
                        iota_s[:].unsqueeze(1).broadcast_to([128, KC, 128]),
                        (sa[:, call * KC:(call + 1) * KC]
                         .unsqueeze(2).broadcast_to([128, KC, 128])),
                        ALU.is_equal)
                if cb < 0:
                    continue
                if cb != prev_cb:
                    flushA(pend, prev_cb)
                    pend = []
                    prev_cb = cb
                pend.append((cur_tile, cur_ind, ck % KC))
            flushA(pend, prev_cb)

            # normalize -> bf16 paired half table (class c at elems c*64)
            NBS = 32   # blocks per normalize slab
            cb0 = 0
            while cb0 < CBH:
                nb_ = min(NBS, CBH - cb0)
                tbf = pAn.tile([128, NBS, H], bf16, tag="nrmb")
                ivb = (iv[:, cb0:cb0 + nb_]
                       .unsqueeze(2).broadcast_to([128, nb_, H]))
                nc.vector.tensor_mul(tbf[:, :nb_, :],
                                     csum_sb[:, cb0:cb0 + nb_, :], ivb)
                nc.scalar.dma_start(
                    bass.AP(chalf, cb0 * 128 * H,
                            [[H, 128], [128 * H, nb_], [1, H]]),
                    tbf[:, :nb_, :])
                cb0 += nb_

        # ================= AllGather halves =================
        groups = [[2 * b, 2 * b + 1] for b in range(n_cores // 2)]
        nc.gpsimd.collective_compute(
            "AllGather", mybir.AluOpType.bypass,
            replica_groups=groups,
            ins=[bass.AP(chalf, 0, [[128, QH], [1, 128]]).opt()],
            outs=[bass.AP(cemb2, SCR * 128, [[128, QT], [1, 128]]).opt()])

        # copy AG output out of the collectives pool (random reads there
        # are ~4x slower) into a plain internal tensor for the gathers
        with tc.tile_pool(name="pcp", bufs=2) as pcp:
            CPR = 4224   # rows per copy slab; RC2 = 198*128
            r0 = 0
            while r0 < RC2:
                rr = min(CPR, RC2 - r0)
                nb_ = rr // 128
                tcp = pcp.tile([128, CPR // 128, 128], bf16, tag="cp")
                nc.scalar.dma_start(
                    tcp[:, :nb_, :],
                    bass.AP(cemb2, r0 * 128,
                            [[128, 128], [128 * 128, nb_], [1, 128]]))
                nc.scalar.dma_start(
                    bass.AP(cembg, r0 * 128,
                            [[128, 128], [128 * 128, nb_], [1, 128]]),
                    tcp[:, :nb_, :])
                r0 += rr

        # ================= stage B + MLP =================
        with tc.tile_pool(name="pw", bufs=1) as pw:
            w1_s = pw.tile([128, 128], bf16)
            b1_s = pw.tile([128, 1], f32)
            st_s = pw.tile([128, 2], bf16)
            ga_s = pw.tile([128, 128], bf16)
            be_s = pw.tile([128, 128], bf16)
            w2_s = pw.tile([128, 2], bf16)
            ident = pw.tile([128, 128], bf16)
            ones2 = pw.tile([128, GROUP], bf16)
            iota_b = pw.tile([128, 128], bf16)
            affr8 = pw.tile([128, GROUP], bf16)
            epsc = pw.tile([128, 1], f32)
            b2c = pw.tile([2, 1], f32)
            ib_t = pw.tile([128, TB // 16], i16)
            sb_t = pw.tile([128, TB // 128], bf16)
            wb_t = pw.tile([128, TB // 128], bf16)
            nc.scalar.dma_start(w1_s[:], w1blk[:, :])
            nc.scalar.dma_start(b1_s[:], b1col[:, :])
            nc.scalar.dma_start(st_s[:], stats_lhsT[:, :])
            nc.scalar.dma_start(ga_s[:], gamma2[:, :])
            nc.scalar.dma_start(be_s[:], beta2[:, :])
            nc.scalar.dma_start(w2_s[:], w2col[:, :])
            nc.scalar.dma_start(ident[:], identd[:, :])
            nc.scalar.dma_start(iota_b[:], iota_d[:, :])
            nc.scalar.dma_start(ib_t[:], gidxB[:, :])
            nc.scalar.dma_start(sb_t[:], segB[:, :])
            nc.scalar.dma_start(wb_t[:], wgtB[:, :])
            nc.vector.memset(ones2[:], 1.0)
            nc.vector.memset(epsc[:], LN_EPS)
            nc.vector.memset(b2c[:], b2v)

            blk_chunks = {}
            for ck, (nb, prt) in enumerate(schedB):
                if nb >= 0:
                    blk_chunks.setdefault(nb, []).append((ck, prt))

            cemb_ap = bass.AP(cembg, 0, [[128, RC2], [1, 128]])
            SLAB = 4096
            slab_chunks = SLAB // 128
            gpc = GROUP // 128

            with tc.tile_pool(name="pm", bufs=2) as pm, \
                 tc.tile_pool(name="pBg", bufs=8) as pBg, \
                 tc.tile_pool(name="pp", bufs=2, space="PSUM") as pp, \
                 tc.tile_pool(name="pp1", bufs=1, space="PSUM") as pp1, \
                 tc.tile_pool(name="ppB", bufs=2, space="PSUM") as ppB:

                gtiles = {}

                def get_gtile(call):
                    if call not in gtiles:
                        t = pBg.tile([128, KC, 128], bf16, tag="gB",
                                     bufs=10)
                        nc.gpsimd.dma_gather(
                            t[:], cemb_ap,
                            ib_t[:, call * TG // 16:(call + 1) * TG // 16],
                            TG, TG, 128, queue_num=call % 4)
                        ind = pBg.tile([128, KC, 128], bf16, tag="indB",
                                       bufs=2)
                        nc.vector.tensor_tensor(
                            ind[:],
                            iota_b[:].unsqueeze(1)
                            .broadcast_to([128, KC, 128]),
                            (sb_t[:, call * KC:(call + 1) * KC]
                             .unsqueeze(2).broadcast_to([128, KC, 128])),
                            ALU.is_equal)
                        indw = pBg.tile([128, KC, 128], bf16, tag="indw",
                                        bufs=12)
                        nc.vector.tensor_mul(
                            indw[:], ind[:],
                            (wb_t[:, call * KC:(call + 1) * KC]
                             .unsqueeze(2).broadcast_to([128, KC, 128])))
                        gtiles[call] = (t, indw)
                    return gtiles[call]

                # last call needed by each block
                blk_last_call = {}
                for nb, cks in blk_chunks.items():
                    blk_last_call[nb] = max(ck for ck, _p in cks) // KC

                import os as _os
                KSTUB = _os.environ.get("KSTUB", "0") == "1"
                n_slab = (NTOK + SLAB - 1) // SLAB
                for s in range(n_slab):
                    t0 = s * SLAB
                    t1 = min(t0 + SLAB, NTOK)
                    ntok = t1 - t0
                    nch = ntok // 128
                    et = pm.tile([128, slab_chunks, H], bf16, tag="embm")
                    xt = pm.tile([128, slab_chunks, H], bf16, tag="xm")
                    lgslab = None
                    if not KSTUB:
                        lgslab = pm.tile([2, (SLAB // GROUP) * GROUP], f32,
                                         tag="lgs")
                    nc.scalar.dma_start(
                        et[:, :nch, :],
                        bass.AP(embN, t0 * H,
                                [[H, 128], [128 * H, nch], [1, H]]))

                    # issue gathers+indicators in groups of 8 calls, then
                    # process the blocks fully covered so far
                    c_lo = min(blk_last_call[t0 // 128 + j]
                               for j in range(nch))
                    c_hi = max(blk_last_call[t0 // 128 + j]
                               for j in range(nch))
                    jdone = 0
                    for cg in range(c_lo, c_hi + 1, 8):
                        for call in range(cg, min(cg + 8, c_hi + 1)):
                            get_gtile(call)
                        if KSTUB:
                            continue
                        cov = min(cg + 8, c_hi + 1) - 1
                        while jdone < nch and \
                                blk_last_call[t0 // 128 + jdone] <= cov:
                            j = jdone
                            nb = t0 // 128 + j
                            chunks = blk_chunks.get(nb, [])
                            ps = ppB.tile([128, 64], f32, tag="psB")
                            nc.tensor.matmul(ps[:], ident[:], et[:, j, :],
                                             start=True, stop=False)
                            nck = len(chunks)
                            for i, (ck, prt) in enumerate(chunks):
                                tl, ind_t = get_gtile(ck // KC)
                                nc.tensor.matmul(
                                    ps[:], ind_t[:, ck % KC, :],
                                    tl[:, ck % KC, 64 * prt:64 * prt + 64],
                                    start=False, stop=(i == nck - 1))
                            nc.scalar.copy(xt[:, j, :], ps[:])
                            jdone += 1

                    # ---- MLP: groups of 4 pairs share one stats chain
                    npr = 0 if KSTUB else ntok // (2 * GROUP)
                    for pg0 in range(0, npr, 3):
                        prs = list(range(pg0, min(pg0 + 3, npr)))
                        pst8 = pp1.tile([128, GROUP], f32, tag="pst8")
                        psq8 = pp1.tile([128, GROUP], f32, tag="psq8")
                        h1s = {}
                        for jl, pr in enumerate(prs):
                            xT = pp.tile([128, GROUP], bf16, tag="mmp")
                            for jj in range(gpc):
                                c0 = 2 * (pr * gpc + jj)
                                nc.tensor.transpose(
                                    xT[:, jj * 128:(jj + 1) * 128],
                                    xt[:, c0:c0 + 2, :], ident[:])
                            xT_sb = pm.tile([128, GROUP], bf16, tag="xTsb",
                                            bufs=3)
                            nc.scalar.copy(xT_sb[:], xT[:])
                            ph = pp.tile([128, GROUP], f32, tag="mmp")
                            nc.tensor.matmul(ph[:], w1_s[:], xT_sb[:])
                            h1 = pm.tile([128, GROUP], bf16, tag="h1",
                                         bufs=6)
                            sq = pm.tile([128, GROUP], bf16, tag="sq",
                                         bufs=3)
                            nc.vector.tensor_scalar(
                                h1[:], ph[:], b1_s[:], None, ALU.add,
                                ALU.bypass)
                            nc.vector.tensor_mul(sq[:], h1[:], h1[:])
                            nc.tensor.matmul(
                                pst8[32 * jl:32 * jl + 2, :], st_s[:], h1[:])
                            nc.tensor.matmul(
                                psq8[32 * jl:32 * jl + 2, :], st_s[:], sq[:])
                            h1s[pr] = h1
                        # stats lhsT carries 1/H: pst8 = mean, psq8 = E[h^2]
                        nrow = 32 * (len(prs) - 1) + 2
                        sm8 = pm.tile([128, GROUP], f32, tag="sm8",
                                      bufs=1)
                        var8 = pm.tile([128, GROUP], f32, tag="var8",
                                       bufs=1)
                        sd8 = pm.tile([128, GROUP], f32, tag="sd8", bufs=1)
                        rstd8 = pm.tile([128, GROUP], f32, tag="rstd8",
                                        bufs=1)
                        rstd8_bf = pm.tile([128, GROUP], bf16, tag="rstd8b",
                                           bufs=1)
                        nc.scalar.copy(sm8[:nrow, :], pst8[:nrow, :])
                        nc.vector.scalar_tensor_tensor(
                            var8[:nrow, :], sm8[:nrow, :], -1.0,
                            sm8[:nrow, :], ALU.mult, ALU.mult)
                        nc.vector.scalar_tensor_tensor(
                            var8[:nrow, :], psq8[:nrow, :], 1.0,
                            var8[:nrow, :], ALU.mult, ALU.add)
                        nc.scalar.activation(sd8[:nrow, :], var8[:nrow, :],
                                             AF.Sqrt, bias=epsc[:nrow, :],
                                             scale=1.0)
                        nc.vector.reciprocal_approx_fast(rstd8[:nrow, :],
                                                         sd8[:nrow, :])
                        nc.scalar.copy(rstd8_bf[:nrow, :], rstd8[:nrow, :])
                        nc.vector.scalar_tensor_tensor(
                            affr8[:nrow, :], sm8[:nrow, :], -1.0,
                            rstd8[:nrow, :], ALU.mult, ALU.mult)
                        for jl, pr in enumerate(prs):
                            g0 = t0 // GROUP + 2 * pr
                            h1 = h1s[pr]
                            pscale = pp1.tile([128, GROUP], f32,
                                              tag="pscale")
                            poff = pp1.tile([128, GROUP], f32, tag="poff")
                            sl = slice(32 * jl, 32 * jl + 2)
                            nc.tensor.matmul(
                                pscale[:], ga_s[sl, :], rstd8_bf[sl, :])
                            nc.tensor.matmul(
                                poff[:], ga_s[sl, :], affr8[sl, :],
                                start=True, stop=False)
                            nc.tensor.matmul(poff[:], be_s[sl, :],
                                             ones2[sl, :],
                                             start=False, stop=True)
                            t1t = pm.tile([128, GROUP], f32, tag="t1t")
                            h3 = pm.tile([128, GROUP], bf16, tag="h3")
                            nc.vector.tensor_mul(t1t[:], h1[:], pscale[:])
                            nc.vector.tensor_add(t1t[:], t1t[:], poff[:])
                            nc.scalar.activation(h3[:], t1t[:], AF.Relu)
                            pL2 = pp1.tile([2, GROUP], f32, tag="pst8")
                            nc.tensor.matmul(pL2[:], w2_s[:], h3[:])
                            nc.scalar.activation(
                                lgslab[:, pr * GROUP:(pr + 1) * GROUP],
                                pL2[:], AF.Identity, bias=b2c[:], scale=1.0)
                    if not KSTUB:
                        nc.scalar.dma_start(
                            bass.AP(out, (t0 // GROUP) * GROUP,
                                    [[GROUP, 2],
                                     [2 * GROUP, ntok // (2 * GROUP)],
                                     [1, GROUP]]),
                            lgslab[:, :ntok // 2])

    nc.compile()
    return nc


def weight_tensors(inputs):
    import ml_dtypes
    bf = ml_dtypes.bfloat16
    W1 = np.asarray(inputs["W1"], dtype=np.float32)
    b1 = np.asarray(inputs["b1"], dtype=np.float32)
    gamma = np.asarray(inputs["gamma"], dtype=np.float32)
    beta = np.asarray(inputs["beta"], dtype=np.float32)
    W2 = np.asarray(inputs["W2"], dtype=np.float32)
    w1blk = np.zeros((128, 128), dtype=np.float32)
    w1blk[:H, :H] = W1
    w1blk[H:, H:] = W1
    b1col = np.concatenate([b1, b1]).reshape(128, 1).astype(np.float32)
    stats = np.zeros((128, 2), dtype=np.float32)
    stats[:H, 0] = 1.0 / H
    stats[H:, 1] = 1.0 / H
    gamma2 = np.zeros((128, 128), dtype=np.float32)
    beta2 = np.zeros((128, 128), dtype=np.float32)
    for base in (0, 32, 64):
        gamma2[base, :H] = gamma
        gamma2[base + 1, H:] = gamma
        beta2[base, :H] = beta
        beta2[base + 1, H:] = beta
    w2col = np.zeros((128, 2), dtype=np.float32)
    w2col[:H, 0] = W2[:, 0]
    w2col[H:, 1] = W2[:, 0]
    iota = np.tile(np.arange(128, dtype=np.float32), (128, 1))
    return dict(
        w1blk=w1blk.astype(bf), b1col=b1col,
        stats_lhsT=stats.astype(bf), gamma2=gamma2.astype(bf),
        beta2=beta2.astype(bf), w2col=w2col.astype(bf),
        identd=np.eye(128, dtype=np.float32).astype(bf),
        iota_d=iota.astype(bf))


def build_in_maps(cfg, inputs, pre, wts, aux):
    import ml_dtypes
    bf = ml_dtypes.bfloat16
    emb_full = np.asarray(inputs["embedding"], dtype=np.float32)
    CH, CBH = cfg["CH"], cfg["CBH"]
    nodes_h, n_h = aux["nodes_h"], aux["n_h"]
    in_maps = []
    for core in range(cfg["n_cores"]):
        b, h = core // 2, core % 2
        d = pre[h]
        stream = emb_full[b][nodes_h[h]].astype(bf)     # [n_h, H] class-sorted
        embN_a = np.zeros((cfg["NTOK"], H), dtype=bf)
        embN_a[:n_h[h]] = stream
        embA_a = np.zeros((cfg["TA"], H), dtype=bf)
        valid = d["rowsrcA"] >= 0
        embA_a[valid] = stream[d["rowsrcA"][valid]]
        invc_h = aux["invc"][h * CH:(h + 1) * CH]
        invc_h = np.pad(invc_h, (0, CBH * 128 - len(invc_h)),
                        constant_values=1.0)
        m = dict(
            embA=embA_a, segA=d["segA"], embN=embN_a,
            gidxB=d["gidxB"], segB=d["segB"], wgtB=d["wgtB"],
            invc_tok=_cols128(invc_h), **wts)
        in_maps.append(m)
    return in_maps


def assemble_out(cfg, aux, results):
    B, N = cfg["B"], cfg["N"]
    nodes_h, n_h = aux["nodes_h"], aux["n_h"]
    npair = cfg["NGRP"] // 2
    out = np.empty((B, N), dtype=np.float32)
    for core in range(cfg["n_cores"]):
        b, h = core // 2, core % 2
        a = np.asarray(results[core]["out"]).reshape(npair, 2, 4, 128)
        o = a.transpose(0, 2, 1, 3).reshape(-1)[:n_h[h]]
        out[b, nodes_h[h]] = o
    return out


def kernel(**inputs):
    emb = np.asarray(inputs["embedding"])
    B, N, _ = emb.shape
    C = int(inputs["num_classes"])
    E = len(np.asarray(inputs["n2c_row"]))
    cfg = make_cfg(B, N, C, E)
    pre, meta, aux = host_prep(cfg, inputs)
    wts = weight_tensors(inputs)
    wvals = dict(b2=float(np.asarray(inputs["b2"]).reshape(-1)[0]))
    nc = build(cfg, meta, wvals)
    in_maps = build_in_maps(cfg, inputs, pre, wts, aux)
    from concourse.bass_utils import run_bass_kernel_spmd
    res = run_bass_kernel_spmd(nc, in_maps,
                               core_ids=list(range(cfg["n_cores"])))
    return assemble_out(cfg, aux, res.results)


# revision 20
# speedup vs baseline: 2.3548x; 1.0244x over previous
"""V3: class-half sharded GNN kernel, gather-free stage A.

Core (b, h) owns batch b and CLASS-half h: the nodes whose own class
(c2n_row) falls in half h.  Stage A needs no AllReduce: each core computes
complete class sums for its half from host-presorted contiguous bf16 rows
(class-block padded) via indicator matmuls, normalizes, and writes a bf16
paired-class half-table; a pairwise AllGather concatenates the halves.
Stage B gathers paired-class rows per edge (dma_gather, deep-buffered so
the ~2.2us/call cadence is not consumer-stalled) and indicator-matmuls
node-context blocks; fused LayerNorm MLP with Rsqrt activation.
"""

import numpy as np

SCR = 128
H = 64
LN_EPS = 1e-5


def _ru(x, m):
    return (x + m - 1) // m * m


def _wrap16(idx):
    n = len(idx)
    n16 = _ru(n, 16)
    a = np.full(n16, -1, dtype=np.int16)
    a[:n] = idx
    a = a.reshape(n16 // 16, 16).T
    return np.tile(a, (8, 1)).copy()


def _cols128(vals, dtype=np.float32):
    n = len(vals)
    return np.asarray(vals, dtype=dtype).reshape(n // 128, 128).T.copy()


def make_cfg(B, N, C, E, tg=1024):
    cfg = dict(B=B, N=N, C=C, E=E, n_cores=2 * B, TG=tg)
    cfg["GROUP"] = 512
    cfg["CPAD"] = _ru(C, 256)
    cfg["CH"] = cfg["CPAD"] // 2          # classes per half
    cfg["QH"] = cfg["CH"] // 2            # paired-class rows per half
    cfg["QT"] = 2 * cfg["QH"]
    cfg["CBH"] = cfg["CH"] // 128         # class blocks per half
    cfg["RC2"] = SCR + cfg["QT"] + SCR    # gather table rows
    assert SCR + cfg["QT"] < 32768
    return cfg


def host_prep(cfg, inputs):
    N, C, CH = cfg["N"], cfg["C"], cfg["CH"]
    TG, CBH = cfg["TG"], cfg["CBH"]
    c2n_row = np.asarray(inputs["c2n_row"]).astype(np.int64)
    n2c_row = np.asarray(inputs["n2c_row"]).astype(np.int64)
    n2c_col = np.asarray(inputs["n2c_col"]).astype(np.int64)

    cnt_c = np.bincount(c2n_row, minlength=C).astype(np.float32)
    invc = (1.0 / np.maximum(cnt_c, 1.0)).astype(np.float32)
    cnt_n = np.bincount(n2c_row, minlength=N).astype(np.float32)
    invn = (1.0 / np.maximum(cnt_n, 1.0)).astype(np.float32)

    # ---------- node streams per class-half
    nodes_h, order_h = {}, {}
    for h in (0, 1):
        sel = np.nonzero((c2n_row >= h * CH) & (c2n_row < (h + 1) * CH))[0]
        order = np.argsort(c2n_row[sel], kind="stable")
        nodes_h[h] = sel[order]           # original node ids, class-sorted
    n_h = {h: len(nodes_h[h]) for h in (0, 1)}
    NTOK = _ru(max(n_h.values()), 1024)
    cfg["NTOK"] = NTOK
    cfg["NGRP"] = NTOK // cfg["GROUP"]
    NBLK = NTOK // 128
    cfg["NBLK"] = NBLK

    # ---------- stage A caps (shared): members per class block
    capsA = np.zeros(CBH, dtype=np.int64)
    for h in (0, 1):
        cls_loc = c2n_row[nodes_h[h]] - h * CH
        capsA = np.maximum(capsA, np.bincount(cls_loc // 128, minlength=CBH))
    capsA = _ru(np.maximum(capsA, 1), 128)
    TA = int(capsA.sum())
    TA_pad = _ru(TA, TG)
    schedA = []
    for cb in range(CBH):
        for k in range(capsA[cb] // 128):
            schedA.append((cb, k == 0))
    for _ in range((TA_pad - TA) // 128):
        schedA.append((-1, False))
    cfg["TA"] = TA_pad

    # ---------- stage B caps: edges per (node block, parity)
    edges_h = {}
    capsB = np.zeros((NBLK, 2), dtype=np.int64)
    for h in (0, 1):
        pos = np.full(N, -1, dtype=np.int64)
        pos[nodes_h[h]] = np.arange(n_h[h])
        sel = np.nonzero(pos[n2c_row] >= 0)[0]
        dst = pos[n2c_row[sel]]
        col = n2c_col[sel]
        par = col % 2
        order = np.lexsort((par, dst // 128))
        edges_h[h] = (dst[order], col[order], par[order])
        nbk = dst[order] // 128
        for prt in (0, 1):
            cnt = np.bincount(nbk[par[order] == prt], minlength=NBLK)
            capsB[:, prt] = np.maximum(capsB[:, prt], cnt)
    capsB = _ru(np.maximum(capsB, 1), 128)
    TBn = int(capsB.sum())
    TB = _ru(TBn, TG)
    cfg["TB"] = TB
    schedB = []
    for nb in range(NBLK):
        for prt in (0, 1):
            for k in range(capsB[nb, prt] // 128):
                schedB.append((nb, prt))
    for _ in range((TB - TBn) // 128):
        schedB.append((-1, 0))

    # ---------- per-half index arrays
    pre = {}
    for h in (0, 1):
        d = {}
        # stage A: padded row placement + segids
        cls_loc = c2n_row[nodes_h[h]] - h * CH
        segA = np.full(cfg["TA"], 255, dtype=np.float32)
        rowsrc = np.full(cfg["TA"], -1, dtype=np.int64)  # index into node stream
        cnts = np.bincount(cls_loc // 128, minlength=CBH)
        starts = np.r_[0, np.cumsum(cnts)]
        base = 0
        for cb in range(CBH):
            nmem = int(cnts[cb])
            s = starts[cb]
            rowsrc[base:base + nmem] = np.arange(s, s + nmem)
            segA[base:base + nmem] = cls_loc[s:s + nmem] - 128 * cb
            base += capsA[cb]
        import ml_dtypes
        d["rowsrcA"] = rowsrc
        d["segA"] = _cols128(segA, ml_dtypes.bfloat16)

        # stage B
        dst, col, par = edges_h[h]
        nbk = dst // 128
        # pad slots gather arbitrary (ignored) rows; spread them to avoid
        # all pads hammering one HBM row
        gidx = SCR + (np.arange(TB, dtype=np.int64) * 97) % cfg["QT"]
        segB = np.full(TB, 255, dtype=np.float32)
        wgtB = np.zeros(TB, dtype=np.float32)
        base = 0
        for nb in range(NBLK):
            for prt in (0, 1):
                m = (nbk == nb) & (par == prt)
                nmem = int(m.sum())
                gidx[base:base + nmem] = SCR + col[m] // 2
                segB[base:base + nmem] = dst[m] - 128 * nb
                wgtB[base:base + nmem] = invn[nodes_h[h][dst[m]]]
                base += capsB[nb, prt]
        import ml_dtypes
        bf = ml_dtypes.bfloat16
        d["gidxB"] = _wrap16(gidx)
        iw = np.zeros((TB, 128), dtype=np.float32)
        val = segB < 255
        iw[np.nonzero(val)[0], segB[val].astype(np.int64)] = wgtB[val]
        d["indwB"] = iw.astype(bf)
        pre[h] = d

    meta = dict(schedA=schedA, schedB=schedB)
    return pre, meta, dict(invc=invc, nodes_h=nodes_h, n_h=n_h)


def build(cfg, meta, wvals):
    from concourse import bass, bacc, tile, mybir

    f32 = mybir.dt.float32
    bf16 = mybir.dt.bfloat16
    i16 = mybir.dt.int16
    AF = mybir.ActivationFunctionType
    ALU = mybir.AluOpType

    CH, QH, QT = cfg["CH"], cfg["QH"], cfg["QT"]
    CBH, RC2 = cfg["CBH"], cfg["RC2"]
    NTOK, TG, GROUP = cfg["NTOK"], cfg["TG"], cfg["GROUP"]
    NBLK, TA, TB = cfg["NBLK"], cfg["TA"], cfg["TB"]
    n_cores = cfg["n_cores"]
    schedA, schedB = meta["schedA"], meta["schedB"]
    KC = TG // 128

    nc = bacc.Bacc("TRN2", target_bir_lowering=False, debug=False,
                   num_devices=n_cores, num_swdge_queues=4)

    embA = nc.dram_tensor("embA", [TA, H], bf16, kind="ExternalInput")
    segA = nc.dram_tensor("segA", [128, TA // 128], bf16, kind="ExternalInput")
    embN = nc.dram_tensor("embN", [NTOK, H], bf16, kind="ExternalInput")
    gidxB = nc.dram_tensor("gidxB", [128, TB // 16], i16, kind="ExternalInput")
    indwB = nc.dram_tensor("indwB", [TB, 128], bf16, kind="ExternalInput")
    invc_tok = nc.dram_tensor("invc_tok", [128, CBH], f32,
                              kind="ExternalInput")
    iota_d = nc.dram_tensor("iota_d", [128, 128], bf16, kind="ExternalInput")
    w1blk = nc.dram_tensor("w1blk", [128, 128], bf16, kind="ExternalInput")
    b1col = nc.dram_tensor("b1col", [128, 1], f32, kind="ExternalInput")
    stats_lhsT = nc.dram_tensor("stats_lhsT", [128, 2], bf16,
                                kind="ExternalInput")
    gamma2 = nc.dram_tensor("gamma2", [128, 128], bf16, kind="ExternalInput")
    beta2 = nc.dram_tensor("beta2", [128, 128], bf16, kind="ExternalInput")
    w2col = nc.dram_tensor("w2col", [128, 2], bf16, kind="ExternalInput")
    identd = nc.dram_tensor("identd", [128, 128], bf16, kind="ExternalInput")
    out = nc.dram_tensor("out", [cfg["NGRP"], GROUP], f32,
                         kind="ExternalOutput")

    chalf = nc.dram_tensor("chalf", [QH, 128], bf16)
    cemb2 = nc.dram_tensor("cemb2", [RC2, 128], bf16)

    b2v = float(wvals["b2"])

    with tile.TileContext(nc) as tc:
        # ================= stage A =================
        with tc.tile_pool(name="pA", bufs=1) as pA, \
             tc.tile_pool(name="pAg", bufs=4) as pAg, \
             tc.tile_pool(name="pAn", bufs=3) as pAn, \
             tc.tile_pool(name="pAp", bufs=4, space="PSUM") as pAp:
            csum_sb = pA.tile([128, CBH, H], f32)
            sa = pA.tile([128, TA // 128], bf16)
            iota_s = pA.tile([128, 128], bf16)
            iv = pA.tile([128, CBH], f32)
            zb = pA.tile([128, SCR], bf16)
            nc.scalar.dma_start(sa[:], segA[:, :])
            nc.scalar.dma_start(iota_s[:], iota_d[:, :])
            nc.scalar.dma_start(iv[:], invc_tok[:, :])
            nc.vector.memset(csum_sb[:], 0.0)
            nc.vector.memset(zb[:], 0.0)
            # zero guard rows of the gather table
            nc.scalar.dma_start(bass.AP(cemb2, 0, [[128, SCR], [1, 128]]),
                                zb[:])

            cur_tile = None
            cur_ind = None
            pend = []
            prev_cb = None

            def flushA(pend, cb):
                if not pend:
                    return
                ps = pAp.tile([128, H], f32, tag="psA")
                for i, (tl, ind_t, kk) in enumerate(pend):
                    nc.tensor.matmul(ps[:], ind_t[:, kk, :], tl[:, kk, :],
                                     start=(i == 0),
                                     stop=(i == len(pend) - 1))
                nc.vector.tensor_add(csum_sb[:, cb, :],
                                     csum_sb[:, cb, :], ps[:])

            for ck, (cb, first) in enumerate(schedA):
                call = ck // KC
                if ck % KC == 0:
                    cur_tile = pAg.tile([128, KC, H], bf16, tag="gA")
                    nc.scalar.dma_start(
                        cur_tile[:],
                        bass.AP(embA, call * TG * H,
                                [[H, 128], [128 * H, KC], [1, H]]))
                    cur_ind = pAg.tile([128, KC, 128], bf16, tag="indA")
                    nc.vector.tensor_tensor(
                        cur_ind[:],
                        iota_s[:].unsqueeze(1).broadcast_to([128, KC, 128]),
                        (sa[:, call * KC:(call + 1) * KC]
                         .unsqueeze(2).broadcast_to([128, KC, 128])),
                        ALU.is_equal)
                if cb < 0:
                    continue
                if cb != prev_cb:
                    flushA(pend, prev_cb)
                    pend = []
                    prev_cb = cb
                pend.append((cur_tile, cur_ind, ck % KC))
            flushA(pend, prev_cb)

            # normalize -> bf16 paired half table (class c at elems c*64)
            NBS = 32   # blocks per normalize slab
            cb0 = 0
            while cb0 < CBH:
                nb_ = min(NBS, CBH - cb0)
                tbf = pAn.tile([128, NBS, H], bf16, tag="nrmb")
                ivb = (iv[:, cb0:cb0 + nb_]
                       .unsqueeze(2).broadcast_to([128, nb_, H]))
                nc.vector.tensor_mul(tbf[:, :nb_, :],
                                     csum_sb[:, cb0:cb0 + nb_, :], ivb)
                nc.scalar.dma_start(
                    bass.AP(chalf, cb0 * 128 * H,
                            [[H, 128], [128 * H, nb_], [1, H]]),
                    tbf[:, :nb_, :])
                cb0 += nb_

        # ================= AllGather halves =================
        groups = [[2 * b, 2 * b + 1] for b in range(n_cores // 2)]
        nc.gpsimd.collective_compute(
            "AllGather", mybir.AluOpType.bypass,
            replica_groups=groups,
            ins=[bass.AP(chalf, 0, [[128, QH], [1, 128]]).opt()],
            outs=[bass.AP(cemb2, SCR * 128, [[128, QT], [1, 128]]).opt()])

        # ================= stage B + MLP =================
        with tc.tile_pool(name="pw", bufs=1) as pw:
            w1_s = pw.tile([128, 128], bf16)
            b1_s = pw.tile([128, 1], f32)
            st_s = pw.tile([128, 2], bf16)
            ga_s = pw.tile([128, 128], bf16)
            be_s = pw.tile([128, 128], bf16)
            w2_s = pw.tile([128, 2], bf16)
            ident = pw.tile([128, 128], bf16)
            ones2 = pw.tile([128, GROUP], bf16)
            affr8 = pw.tile([128, GROUP], bf16)
            epsc = pw.tile([128, 1], f32)
            b2c = pw.tile([2, 1], f32)
            ib_t = pw.tile([128, TB // 16], i16)
            nc.scalar.dma_start(w1_s[:], w1blk[:, :])
            nc.scalar.dma_start(b1_s[:], b1col[:, :])
            nc.scalar.dma_start(st_s[:], stats_lhsT[:, :])
            nc.scalar.dma_start(ga_s[:], gamma2[:, :])
            nc.scalar.dma_start(be_s[:], beta2[:, :])
            nc.scalar.dma_start(w2_s[:], w2col[:, :])
            nc.scalar.dma_start(ident[:], identd[:, :])
            nc.scalar.dma_start(ib_t[:], gidxB[:, :])
            nc.vector.memset(ones2[:], 1.0)
            nc.vector.memset(epsc[:], LN_EPS)
            nc.vector.memset(b2c[:], b2v)

            blk_chunks = {}
            for ck, (nb, prt) in enumerate(schedB):
                if nb >= 0:
                    blk_chunks.setdefault(nb, []).append((ck, prt))

            cemb_ap = bass.AP(cemb2, 0, [[128, RC2], [1, 128]])
            SLAB = 4096
            slab_chunks = SLAB // 128
            gpc = GROUP // 128

            with tc.tile_pool(name="pm", bufs=2) as pm, \
                 tc.tile_pool(name="pBg", bufs=8) as pBg, \
                 tc.tile_pool(name="pp", bufs=2, space="PSUM") as pp, \
                 tc.tile_pool(name="pp1", bufs=1, space="PSUM") as pp1, \
                 tc.tile_pool(name="ppB", bufs=2, space="PSUM") as ppB:

                gtiles = {}

                def get_gtile(call):
                    if call not in gtiles:
                        t = pBg.tile([128, KC, 128], bf16, tag="gB",
                                     bufs=10)
                        nc.gpsimd.dma_gather(
                            t[:], cemb_ap,
                            ib_t[:, call * TG // 16:(call + 1) * TG // 16],
                            TG, TG, 128, queue_num=call % 4)
                        indw = pBg.tile([128, KC, 128], bf16, tag="indw",
                                        bufs=12)
                        nc.scalar.dma_start(
                            indw[:],
                            bass.AP(indwB, call * TG * 128,
                                    [[128, 128], [128 * 128, KC], [1, 128]]))
                        gtiles[call] = (t, indw)
                    return gtiles[call]

                # last call needed by each block
                blk_last_call = {}
                for nb, cks in blk_chunks.items():
                    blk_last_call[nb] = max(ck for ck, _p in cks) // KC

                import os as _os
                KSTUB = _os.environ.get("KSTUB", "0") == "1"
                n_slab = (NTOK + SLAB - 1) // SLAB
                for s in range(n_slab):
                    t0 = s * SLAB
                    t1 = min(t0 + SLAB, NTOK)
                    ntok = t1 - t0
                    nch = ntok // 128
                    et = pm.tile([128, slab_chunks, H], bf16, tag="embm")
                    xt = pm.tile([128, slab_chunks, H], bf16, tag="xm")
                    lgslab = None
                    if not KSTUB:
                        lgslab = pm.tile([2, (SLAB // GROUP) * GROUP], f32,
                                         tag="lgs")
                    nc.scalar.dma_start(
                        et[:, :nch, :],
                        bass.AP(embN, t0 * H,
                                [[H, 128], [128 * H, nch], [1, H]]))

                    # issue gathers+indicators in groups of 8 calls, then
                    # process the blocks fully covered so far
                    c_lo = min(blk_last_call[t0 // 128 + j]
                               for j in range(nch))
                    c_hi = max(blk_last_call[t0 // 128 + j]
                               for j in range(nch))
                    jdone = 0
                    for cg in range(c_lo, c_hi + 1, 8):
                        for call in range(cg, min(cg + 8, c_hi + 1)):
                            get_gtile(call)
                        if KSTUB:
                            continue
                        cov = min(cg + 8, c_hi + 1) - 1
                        while jdone < nch and \
                                blk_last_call[t0 // 128 + jdone] <= cov:
                            j = jdone
                            nb = t0 // 128 + j
                            chunks = blk_chunks.get(nb, [])
                            ps = ppB.tile([128, 64], f32, tag="psB")
                            nc.tensor.matmul(ps[:], ident[:], et[:, j, :],
                                             start=True, stop=False)
                            nck = len(chunks)
                            for i, (ck, prt) in enumerate(chunks):
                                tl, ind_t = get_gtile(ck // KC)
                                nc.tensor.matmul(
                                    ps[:], ind_t[:, ck % KC, :],
                                    tl[:, ck % KC, 64 * prt:64 * prt + 64],
                                    start=False, stop=(i == nck - 1))
                            nc.scalar.copy(xt[:, j, :], ps[:])
                            jdone += 1

                    # ---- MLP: groups of 4 pairs share one stats chain
                    npr = 0 if KSTUB else ntok // (2 * GROUP)
                    for pg0 in range(0, npr, 3):
                        prs = list(range(pg0, min(pg0 + 3, npr)))
                        pst8 = pp1.tile([128, GROUP], f32, tag="pst8")
                        psq8 = pp1.tile([128, GROUP], f32, tag="psq8")
                        h1s = {}
                        for jl, pr in enumerate(prs):
                            xT = pp.tile([128, GROUP], bf16, tag="mmp")
                            for jj in range(gpc):
                                c0 = 2 * (pr * gpc + jj)
                                nc.tensor.transpose(
                                    xT[:, jj * 128:(jj + 1) * 128],
                                    xt[:, c0:c0 + 2, :], ident[:])
                            xT_sb = pm.tile([128, GROUP], bf16, tag="xTsb",
                                            bufs=3)
                            nc.scalar.copy(xT_sb[:], xT[:])
                            ph = pp.tile([128, GROUP], f32, tag="mmp")
                            nc.tensor.matmul(ph[:], w1_s[:], xT_sb[:])
                            h1 = pm.tile([128, GROUP], bf16, tag="h1",
                                         bufs=6)
                            sq = pm.tile([128, GROUP], bf16, tag="sq",
                                         bufs=3)
                            nc.vector.tensor_scalar(
                                h1[:], ph[:], b1_s[:], None, ALU.add,
                                ALU.bypass)
                            nc.vector.tensor_mul(sq[:], h1[:], h1[:])
                            nc.tensor.matmul(
                                pst8[32 * jl:32 * jl + 2, :], st_s[:], h1[:])
                            nc.tensor.matmul(
                                psq8[32 * jl:32 * jl + 2, :], st_s[:], sq[:])
                            h1s[pr] = h1
                        # stats lhsT carries 1/H: pst8 = mean, psq8 = E[h^2]
                        nrow = 32 * (len(prs) - 1) + 2
                        sm8 = pm.tile([128, GROUP], f32, tag="sm8",
                                      bufs=1)
                        var8 = pm.tile([128, GROUP], f32, tag="var8",
                                       bufs=1)
                        sd8 = pm.tile([128, GROUP], f32, tag="sd8", bufs=1)
                        rstd8 = pm.tile([128, GROUP], f32, tag="rstd8",
                                        bufs=1)
                        rstd8_bf = pm.tile([128, GROUP], bf16, tag="rstd8b",
                                           bufs=1)
                        nc.scalar.copy(sm8[:nrow, :], pst8[:nrow, :])
                        nc.vector.scalar_tensor_tensor(
                            var8[:nrow, :], sm8[:nrow, :], -1.0,
                            sm8[:nrow, :], ALU.mult, ALU.mult)
                        nc.vector.scalar_tensor_tensor(
                            var8[:nrow, :], psq8[:nrow, :], 1.0,
                            var8[:nrow, :], ALU.mult, ALU.add)
                        nc.scalar.activation(sd8[:nrow, :], var8[:nrow, :],
                                             AF.Sqrt, bias=epsc[:nrow, :],
                                             scale=1.0)
                        nc.vector.reciprocal_approx_fast(rstd8[:nrow, :],
                                                         sd8[:nrow, :])
                        nc.scalar.copy(rstd8_bf[:nrow, :], rstd8[:nrow, :])
                        nc.vector.scalar_tensor_tensor(
                            affr8[:nrow, :], sm8[:nrow, :], -1.0,
                            rstd8[:nrow, :], ALU.mult, ALU.mult)
                        for jl, pr in enumerate(prs):
                            g0 = t0 // GROUP + 2 * pr
                            h1 = h1s[pr]
                            pscale = pp1.tile([128, GROUP], f32,
                                              tag="pscale")
                            poff = pp1.tile([128, GROUP], f32, tag="poff")
                            sl = slice(32 * jl, 32 * jl + 2)
                            nc.tensor.matmul(
                                pscale[:], ga_s[sl, :], rstd8_bf[sl, :])
                            nc.tensor.matmul(
                                poff[:], ga_s[sl, :], affr8[sl, :],
                                start=True, stop=False)
                            nc.tensor.matmul(poff[:], be_s[sl, :],
                                             ones2[sl, :],
                                             start=False, stop=True)
                            t1t = pm.tile([128, GROUP], f32, tag="t1t")
                            h3 = pm.tile([128, GROUP], bf16, tag="h3")
                            nc.vector.tensor_mul(t1t[:], h1[:], pscale[:])
                            nc.vector.tensor_add(t1t[:], t1t[:], poff[:])
                            nc.scalar.activation(h3[:], t1t[:], AF.Relu)
                            pL2 = pp1.tile([2, GROUP], f32, tag="pst8")
                            nc.tensor.matmul(pL2[:], w2_s[:], h3[:])
                            nc.scalar.activation(
                                lgslab[:, pr * GROUP:(pr + 1) * GROUP],
                                pL2[:], AF.Identity, bias=b2c[:], scale=1.0)
                    if not KSTUB:
                        nc.scalar.dma_start(
                            bass.AP(out, (t0 // GROUP) * GROUP,
                                    [[GROUP, 2],
                                     [2 * GROUP, ntok // (2 * GROUP)],
                                     [1, GROUP]]),
                            lgslab[:, :ntok // 2])

    nc.compile()
    return nc


def weight_tensors(inputs):
    import ml_dtypes
    bf = ml_dtypes.bfloat16
    W1 = np.asarray(inputs["W1"], dtype=np.float32)
    b1 = np.asarray(inputs["b1"], dtype=np.float32)
    gamma = np.asarray(inputs["gamma"], dtype=np.float32)
    beta = np.asarray(inputs["beta"], dtype=np.float32)
    W2 = np.asarray(inputs["W2"], dtype=np.float32)
    w1blk = np.zeros((128, 128), dtype=np.float32)
    w1blk[:H, :H] = W1
    w1blk[H:, H:] = W1
    b1col = np.concatenate([b1, b1]).reshape(128, 1).astype(np.float32)
    stats = np.zeros((128, 2), dtype=np.float32)
    stats[:H, 0] = 1.0 / H
    stats[H:, 1] = 1.0 / H
    gamma2 = np.zeros((128, 128), dtype=np.float32)
    beta2 = np.zeros((128, 128), dtype=np.float32)
    for base in (0, 32, 64):
        gamma2[base, :H] = gamma
        gamma2[base + 1, H:] = gamma
        beta2[base, :H] = beta
        beta2[base + 1, H:] = beta
    w2col = np.zeros((128, 2), dtype=np.float32)
    w2col[:H, 0] = W2[:, 0]
    w2col[H:, 1] = W2[:, 0]
    iota = np.tile(np.arange(128, dtype=np.float32), (128, 1))
    return dict(
        w1blk=w1blk.astype(bf), b1col=b1col,
        stats_lhsT=stats.astype(bf), gamma2=gamma2.astype(bf),
        beta2=beta2.astype(bf), w2col=w2col.astype(bf),
        identd=np.eye(128, dtype=np.float32).astype(bf),
        iota_d=iota.astype(bf))


def build_in_maps(cfg, inputs, pre, wts, aux):
    import ml_dtypes
    bf = ml_dtypes.bfloat16
    emb_full = np.asarray(inputs["embedding"], dtype=np.float32)
    CH, CBH = cfg["CH"], cfg["CBH"]
    nodes_h, n_h = aux["nodes_h"], aux["n_h"]
    in_maps = []
    for core in range(cfg["n_cores"]):
        b, h = core // 2, core % 2
        d = pre[h]
        stream = emb_full[b][nodes_h[h]].astype(bf)     # [n_h, H] class-sorted
        embN_a = np.zeros((cfg["NTOK"], H), dtype=bf)
        embN_a[:n_h[h]] = stream
        embA_a = np.zeros((cfg["TA"], H), dtype=bf)
        valid = d["rowsrcA"] >= 0
        embA_a[valid] = stream[d["rowsrcA"][valid]]
        invc_h = aux["invc"][h * CH:(h + 1) * CH]
        invc_h = np.pad(invc_h, (0, CBH * 128 - len(invc_h)),
                        constant_values=1.0)
        m = dict(
            embA=embA_a, segA=d["segA"], embN=embN_a,
            gidxB=d["gidxB"], indwB=d["indwB"],
            invc_tok=_cols128(invc_h), **wts)
        in_maps.append(m)
    return in_maps


def assemble_out(cfg, aux, results):
    B, N = cfg["B"], cfg["N"]
    nodes_h, n_h = aux["nodes_h"], aux["n_h"]
    npair = cfg["NGRP"] // 2
    out = np.empty((B, N), dtype=np.float32)
    for core in range(cfg["n_cores"]):
        b, h = core // 2, core % 2
        a = np.asarray(results[core]["out"]).reshape(npair, 2, 4, 128)
        o = a.transpose(0, 2, 1, 3).reshape(-1)[:n_h[h]]
        out[b, nodes_h[h]] = o
    return out


def kernel(**inputs):
    emb = np.asarray(inputs["embedding"])
    B, N, _ = emb.shape
    C = int(inputs["num_classes"])
    E = len(np.asarray(inputs["n2c_row"]))
    cfg = make_cfg(B, N, C, E)
    pre, meta, aux = host_prep(cfg, inputs)
    wts = weight_tensors(inputs)
    wvals = dict(b2=float(np.asarray(inputs["b2"]).reshape(-1)[0]))
    nc = build(cfg, meta, wvals)
    in_maps = build_in_maps(cfg, inputs, pre, wts, aux)
    from concourse.bass_utils import run_bass_kernel_spmd
    res = run_bass_kernel_spmd(nc, in_maps,
                               core_ids=list(range(cfg["n_cores"])))
    return assemble_out(cfg, aux, res.results)
